# revision 1
# baseline (speedup 1.0000x reference)
"""DirectedEdgeConv (gnn_message_passing) Trainium2 kernel, 8-core SPMD.

out[e] = leaky_relu(edge_attr[e] @ Wself^T + b
                    + T_in[src[e]] + T_out[dst[e]], 0.2)
where T_in  = scatter_mean(edge_attr, dst) @ Win^T + b   [node table]
      T_out = scatter_mean(edge_attr, src) @ Wout^T      [node table]

Sharding strategy (graph partitioning):
  Phase A (node-sharded): core c owns nodes [c*NPC, (c+1)*NPC). The host
  groups the edge rows by (owning core, 128-node block) of their
  dst (resp. src) endpoint; each core streams its groups contiguously
  and segment-sums them via one-hot matmuls accumulated in PSUM, then
  scales by 1/cnt and applies the (linear) weight at node level. The
  self bias is baked into T_in.
  Phase B: AllGather the per-core [NBP, 128] table slices -> full
  [8*NBP, 128] tables on every core.
  Phase C (edge-sharded): core c streams its contiguous 75k-edge shard,
  computes h_self via PE (transpose + matmul), gathers T_in[src] /
  T_out[dst] rows with dma_gather (int16 indices; edges are grouped by
  (src-half, dst-half) of the table so indices fit 15 bits), adds and
  applies leaky relu.
"""

import os
import sys

sys.path.insert(0, "/opt/trn_rl_repo")

VARIANT = os.environ.get("KVARIANT", "barrier")

import numpy as np

import concourse.bacc as bacc
import concourse.bass as bass
import concourse.mybir as mybir
import concourse.tile as tile
from concourse import library_config
from concourse.bass_utils import run_bass_kernel_spmd
from concourse.masks import make_identity

P = 128
D = 128
C = 8  # cores
HALF = 32768  # int16 index capacity per dma_gather base

F32 = mybir.dt.float32
I16 = mybir.dt.int16

SUP = P * 12  # edges per supertile (KC=12)
KC = 12


def _cfg_full():
    return dict(E=600000, N=50000)


def _derive(cfg):
    E, N = cfg["E"], cfg["N"]
    assert N % C == 0 and E % C == 0
    NPC = N // C
    NB = (NPC + P - 1) // P
    NBP = NB * P
    EPC = E // C
    return NPC, NB, NBP, EPC


def build_kernel(cfg, KA_dst, KA_src, NS):
    """NS: list of 4 supertile counts per (src_hi*2+dst_hi) group."""
    E, N = cfg["E"], cfg["N"]
    NPC, NB, NBP, EPC = _derive(cfg)
    TROWS = C * NBP
    NSUP = sum(NS)
    SIDX = SUP // 16  # int16 idx tile free dim

    nc = bacc.Bacc(None, target_bir_lowering=False, debug=False)

    # ---- I/O ----
    agat_d = nc.dram_tensor("agat_d", [NB, P, KA_dst * D], F32, kind="ExternalInput")
    va_d = nc.dram_tensor("va_d", [NB, P, KA_dst], F32, kind="ExternalInput")
    agat_s = nc.dram_tensor("agat_s", [NB, P, KA_src * D], F32, kind="ExternalInput")
    va_s = nc.dram_tensor("va_s", [NB, P, KA_src], F32, kind="ExternalInput")
    invc_d = nc.dram_tensor("invc_d", [P, NB], F32, kind="ExternalInput")
    invc_s = nc.dram_tensor("invc_s", [P, NB], F32, kind="ExternalInput")
    xshard = nc.dram_tensor("xshard", [NSUP * P, KC * D], F32, kind="ExternalInput")
    gidx_in = nc.dram_tensor("gidx_in", [NSUP, P, SIDX], I16, kind="ExternalInput")
    gidx_out = nc.dram_tensor("gidx_out", [NSUP, P, SIDX], I16, kind="ExternalInput")
    wself = nc.dram_tensor("wself", [D, D], F32, kind="ExternalInput")
    win = nc.dram_tensor("win", [D, D], F32, kind="ExternalInput")
    wout = nc.dram_tensor("wout", [D, D], F32, kind="ExternalInput")
    bbc = nc.dram_tensor("bbc", [P, D], F32, kind="ExternalInput")
    iota_in = nc.dram_tensor("iota", [P, P], F32, kind="ExternalInput")
    y = nc.dram_tensor("y", [NSUP * P, KC * D], F32, kind="ExternalOutput")
    tdum = (
        nc.dram_tensor("tdummy", [TROWS, D], F32, kind="ExternalInput")
        if VARIANT == "gather_ext" else None
    )

    with tile.TileContext(nc) as tc:
        with (
            tc.tile_pool(name="const", bufs=1) as cpool,
            tc.tile_pool(name="sbuf", bufs=3) as pool,
            tc.tile_pool(name="small", bufs=4) as spool,
            tc.tile_pool(name="psum", bufs=2, space="PSUM") as psum,
            tc.tile_pool(name="dram", bufs=1, space="DRAM") as dram,
        ):
            nc.gpsimd.load_library(library_config.mlp)
            # constants
            ident = cpool.tile([P, P], F32)
            make_identity(nc, ident[:])
            iota_t = cpool.tile([P, P], F32)
            nc.sync.dma_start(out=iota_t[:], in_=iota_in[:])
            wself_t = cpool.tile([D, D], F32)
            nc.sync.dma_start(out=wself_t[:], in_=wself[:])
            win_t = cpool.tile([D, D], F32)
            nc.sync.dma_start(out=win_t[:], in_=win[:])
            wout_t = cpool.tile([D, D], F32)
            nc.sync.dma_start(out=wout_t[:], in_=wout[:])
            bbc_t = cpool.tile([P, D], F32)
            nc.sync.dma_start(out=bbc_t[:], in_=bbc[:])
            invc_d_t = cpool.tile([P, NB], F32)
            nc.sync.dma_start(out=invc_d_t[:], in_=invc_d[:])
            invc_s_t = cpool.tile([P, NB], F32)
            nc.sync.dma_start(out=invc_s_t[:], in_=invc_s[:])

            # collective dram buffers
            cc_in_d = dram.tile([NBP, D], F32)
            cc_out_d = dram.tile([TROWS, D], F32)
            cc_in_s = dram.tile([NBP, D], F32)
            cc_out_s = dram.tile([TROWS, D], F32)

            # ---- Phase A ----
            def phase_a(agat, va, KA, invc_t, w_t, cc_in, add_bias):
                for b in range(NB):
                    valt = spool.tile([P, KA], F32, tag="aval")
                    nc.sync.dma_start(out=valt[:], in_=va[b])
                    gat = pool.tile([P, KA * D], F32, tag="agather")
                    nc.sync.dma_start(out=gat[:], in_=agat[b])
                    ps = psum.tile([P, D], F32, tag="pA")
                    for j in range(KA):
                        oh = spool.tile([P, P], F32, tag="oh")
                        nc.vector.tensor_scalar(
                            oh[:], iota_t[:], valt[:, j : j + 1], None,
                            mybir.AluOpType.is_equal,
                        )
                        nc.tensor.matmul(
                            ps[:], oh[:], gat[:, j * D : (j + 1) * D],
                            start=(j == 0), stop=(j == KA - 1),
                        )
                    means = spool.tile([P, D], F32, tag="means")
                    nc.vector.tensor_scalar(
                        means[:], ps[:], invc_t[:, b : b + 1], None,
                        mybir.AluOpType.mult,
                    )
                    pst = psum.tile([P, D], F32, tag="pB")
                    nc.tensor.transpose(pst[:], means[:], ident[:])
                    meansT = spool.tile([P, D], F32, tag="meansT")
                    nc.scalar.copy(out=meansT[:], in_=pst[:])
                    psT = psum.tile([P, D], F32, tag="pC")
                    nc.tensor.matmul(psT[:], meansT[:], w_t[:], start=True, stop=True)
                    tt = spool.tile([P, D], F32, tag="tt")
                    if add_bias:
                        nc.vector.tensor_add(tt[:], psT[:], bbc_t[:])
                    else:
                        nc.scalar.copy(out=tt[:], in_=psT[:])
                    nc.sync.dma_start(out=cc_in[b * P : (b + 1) * P, :], in_=tt[:])

            phase_a(agat_d, va_d, KA_dst, invc_d_t, win_t, cc_in_d, True)
            nc.gpsimd.collective_compute(
                "AllGather", mybir.AluOpType.bypass,
                replica_groups=[list(range(C))],
                ins=[cc_in_d.opt()], outs=[cc_out_d.opt()],
            )
            phase_a(agat_s, va_s, KA_src, invc_s_t, wout_t, cc_in_s, False)
            nc.gpsimd.collective_compute(
                "AllGather", mybir.AluOpType.bypass,
                replica_groups=[list(range(C))],
                ins=[cc_in_s.opt()], outs=[cc_out_s.opt()],
            )

            if VARIANT != "nobarrier":
                tc.strict_bb_all_engine_barrier()

            # ---- Phase C ----
            def tbl_slice(cc_out, hi):
                if VARIANT == "gather_ext":
                    cc_out = tdum
                base = hi * HALF
                size = min(HALF, TROWS - base)
                return cc_out[base : base + size, :]

            s_global = 0
            for g in range(4):
                src_hi, dst_hi = g >> 1, g & 1
                for _ in range(NS[g]):
                    s = s_global
                    s_global += 1
                    sidx = spool.tile([P, SIDX], I16, tag="sidx")
                    nc.sync.dma_start(out=sidx[:], in_=gidx_in[s])
                    didx = spool.tile([P, SIDX], I16, tag="didx")
                    nc.sync.dma_start(out=didx[:], in_=gidx_out[s])
                    xt = pool.tile([P, KC * D], F32, tag="xt")
                    nc.sync.dma_start(out=xt[:], in_=xshard[s * P : (s + 1) * P, :])
                    gi = pool.tile([P, KC * D], F32, tag="gi")
                    go = pool.tile([P, KC * D], F32, tag="go")
                    if VARIANT == "nogather":
                        nc.vector.memset(gi[:], 0.0)
                        nc.vector.memset(go[:], 0.0)
                    else:
                        nc.gpsimd.dma_gather(
                            out_ap=gi[:].rearrange("p (j d) -> p j d", j=KC),
                            in_ap=tbl_slice(cc_out_d, src_hi),
                            idxs_ap=sidx[:],
                            num_idxs=SUP, num_idxs_reg=SUP, elem_size=D,
                            single_packet=False,
                        )
                        nc.gpsimd.dma_gather(
                            out_ap=go[:].rearrange("p (j d) -> p j d", j=KC),
                            in_ap=tbl_slice(cc_out_s, dst_hi),
                            idxs_ap=didx[:],
                            num_idxs=SUP, num_idxs_reg=SUP, elem_size=D,
                            single_packet=False,
                        )
                    yo = pool.tile([P, KC * D], F32, tag="yo")
                    for j in range(KC):
                        sl = slice(j * D, (j + 1) * D)
                        psx = psum.tile([P, D], F32, tag="pA")
                        nc.tensor.transpose(psx[:], xt[:, sl], ident[:])
                        xT = spool.tile([P, D], F32, tag="xT")
                        nc.scalar.copy(out=xT[:], in_=psx[:])
                        psh = psum.tile([P, D], F32, tag="pB")
                        nc.tensor.matmul(
                            psh[:], xT[:], wself_t[:], start=True, stop=True
                        )
                        s1 = spool.tile([P, D], F32, tag="s1")
                        nc.vector.tensor_add(s1[:], gi[:, sl], go[:, sl])
                        s2 = spool.tile([P, D], F32, tag="s2")
                        nc.vector.tensor_add(s2[:], psh[:], s1[:])
                        t1 = spool.tile([P, D], F32, tag="t1")
                        nc.scalar.mul(out=t1[:], in_=s2[:], mul=0.2)
                        nc.vector.tensor_max(yo[:, sl], s2[:], t1[:])
                    nc.sync.dma_start(out=y[s * P : (s + 1) * P, :], in_=yo[:])

    nc.compile()
    return nc


def prepare_inputs(cfg, edge_attr, edge_index, W_self_w, W_self_b, W_in_w, W_out_w):
    """Host-side sharding / graph partitioning. Returns (params, in_maps, post)."""
    E, N = cfg["E"], cfg["N"]
    NPC, NB, NBP, EPC = _derive(cfg)
    TROWS = C * NBP

    edge_attr = np.ascontiguousarray(edge_attr, dtype=np.float32)
    src = np.asarray(edge_index[0], dtype=np.int64)
    dst = np.asarray(edge_index[1], dtype=np.int64)

    wself = np.ascontiguousarray(np.asarray(W_self_w, np.float32).T)
    win = np.ascontiguousarray(np.asarray(W_in_w, np.float32).T)
    wout = np.ascontiguousarray(np.asarray(W_out_w, np.float32).T)
    bbc = np.tile(np.asarray(W_self_b, dtype=np.float32)[None, :], (P, 1))
    iota = np.tile(np.arange(P, dtype=np.float32)[None, :], (P, 1))

    # ---- phase A: group edge rows by (core, block) of endpoint ----
    def build_a(node_of_edge):
        core = node_of_edge // NPC
        local = node_of_edge - core * NPC
        blk = local >> 7
        inblk = (local & 127).astype(np.float32)
        key = (core * NB + blk).astype(np.int64)
        order = np.argsort(key, kind="stable")
        cnts = np.bincount(key, minlength=C * NB)
        KA = max(1, int(np.ceil(cnts.max() / P)))
        starts = np.zeros(C * NB, dtype=np.int64)
        np.cumsum(cnts[:-1], out=starts[1:])
        pos = np.arange(E, dtype=np.int64) - starts[key[order]]
        slot = key[order] * (P * KA) + pos  # flat (group, p*KA+j)
        agat = np.zeros((C * NB * P * KA, D), dtype=np.float32)
        agat[slot] = edge_attr[order]
        agat = agat.reshape(C, NB, P, KA * D)
        va = np.full((C * NB * P * KA), -1.0, dtype=np.float32)
        va[slot] = inblk[order]
        va = va.reshape(C, NB, P, KA)
        cnt_node = np.bincount(node_of_edge, minlength=N).astype(np.float32)
        inv = 1.0 / np.maximum(cnt_node, 1.0)
        inv_pad = np.zeros((C, NBP), dtype=np.float32)
        inv_pad[:, :NPC] = inv.reshape(C, NPC)
        invc = np.ascontiguousarray(inv_pad.reshape(C, NB, P).transpose(0, 2, 1))
        return KA, agat, va, invc

    KA_dst, agat_d, va_d, invc_d = build_a(dst)
    KA_src, agat_s, va_s, invc_s = build_a(src)

    # ---- phase C: 4-way (src_hi, dst_hi) grouping per core ----
    trow = lambda n: (n // NPC) * NBP + (n % NPC)
    src_rows = trow(src)
    dst_rows = trow(dst)
    grp = (src_rows >= HALF) * 2 + (dst_rows >= HALF)

    percore = []
    for c in range(C):
        lo, hi = c * EPC, (c + 1) * EPC
        g = grp[lo:hi]
        order = np.argsort(g, kind="stable")  # local edge order, grouped
        gcnt = np.bincount(g, minlength=4)
        percore.append((order, gcnt))
    NS = [
        max(1 if max(pc[1][g] for pc in percore) > 0 else 0,
            int(np.ceil(max(pc[1][g] for pc in percore) / SUP)))
        for g in range(4)
    ]
    NSUP = sum(NS)

    def wrap_idx(vals):
        # vals: [SUP] int -> [P, SIDX] int16 (16-partition wrap, replicated)
        S = SUP // 16
        t = np.zeros((16, S), dtype=np.int16)
        t[np.arange(SUP) % 16, np.arange(SUP) // 16] = vals.astype(np.int16)
        return np.tile(t, (8, 1))

    in_maps = []
    inv_perms = []
    for c in range(C):
        lo = c * EPC
        order, gcnt = percore[c]
        # slot list: for each group g, its edges then pad to NS[g]*SUP slots
        slot_edge = np.full(NSUP * SUP, -1, dtype=np.int64)  # local edge id or -1
        off = 0
        gstart = np.zeros(5, dtype=np.int64)
        np.cumsum(gcnt, out=gstart[1:])
        for g in range(4):
            cnt = gcnt[g]
            slot_edge[off : off + cnt] = order[gstart[g] : gstart[g] + cnt]
            off += NS[g] * SUP
        valid = slot_edge >= 0
        le = np.where(valid, slot_edge, 0)
        ge = le + lo  # global edge id (pad -> lo, masked later)
        # xshard: supertile s, slot gpos=j*128+p -> [s*P+p, j*D:(j+1)*D]
        xs = np.where(valid[:, None], edge_attr[ge], 0).astype(np.float32)
        xs = xs.reshape(NSUP, KC, P, D).transpose(0, 2, 1, 3).reshape(NSUP * P, KC * D)
        # gather indices (rebased per group)
        si = src_rows[ge].astype(np.int64)
        di = dst_rows[ge].astype(np.int64)
        off = 0
        for g in range(4):
            sl = slice(off, off + NS[g] * SUP)
            si[sl] -= (g >> 1) * HALF
            di[sl] -= (g & 1) * HALF
            off += NS[g] * SUP
        si = np.where(valid, si, 0)
        di = np.where(valid, di, 0)
        gin = np.stack([wrap_idx(si[s * SUP : (s + 1) * SUP]) for s in range(NSUP)])
        gout = np.stack([wrap_idx(di[s * SUP : (s + 1) * SUP]) for s in range(NSUP)])
        in_maps.append(
            dict(
                agat_d=agat_d[c], va_d=va_d[c], agat_s=agat_s[c], va_s=va_s[c],
                invc_d=invc_d[c], invc_s=invc_s[c],
                xshard=xs, gidx_in=gin, gidx_out=gout,
                wself=wself, win=win, wout=wout, bbc=bbc, iota=iota,
            )
        )
        inv_perms.append((slot_edge, valid))

    def postprocess(results):
        full = np.empty((E, D), dtype=np.float32)
        for c in range(C):
            yv = results[c]["y"].reshape(NSUP, P, KC, D).transpose(0, 2, 1, 3)
            yv = yv.reshape(NSUP * SUP, D)
            slot_edge, valid = inv_perms[c]
            full[c * EPC + slot_edge[valid]] = yv[valid]
        return full

    params = (KA_dst, KA_src, tuple(NS))
    return params, in_maps, postprocess


_NC_CACHE = {}


def run(cfg, inputs, trace=False, trace_kwargs=None):
    params, in_maps, post = prepare_inputs(
        cfg,
        inputs["edge_attr"],
        inputs["edge_index"],
        inputs["W_self_w"],
        inputs["W_self_b"],
        inputs["W_in_w"],
        inputs["W_out_w"],
    )
    key = (tuple(sorted(cfg.items())), params)
    if key not in _NC_CACHE:
        _NC_CACHE[key] = build_kernel(cfg, params[0], params[1], list(params[2]))
    nc = _NC_CACHE[key]
    kw = {}
    if trace:
        kw["trace"] = True
        if trace_kwargs:
            kw.update(trace_kwargs)
    res = run_bass_kernel_spmd(nc, in_maps, core_ids=list(range(C)), **kw)
    return post(res.results), res


def kernel(**inputs) -> np.ndarray:
    out, _ = run(_cfg_full(), inputs)
    return out.astype(np.float32)



# revision 2
# speedup vs baseline: 1.0892x; 1.0892x over previous
"""DirectedEdgeConv Trainium2 kernel, 8-core SPMD — v2 (descriptor-free).

out[e] = prelu(x[e] @ Wself^T + b + T_in[src[e]] + T_out[dst[e]], 0.2)
  T_in  = scatter_mean(x, dst) @ Win^T    (bias folded into prelu's bias)
  T_out = scatter_mean(x, src) @ Wout^T

Sharding: edges assigned to core own(src). Per core:
  A1: stream dst-owner-grouped edges (dense KA buckets per 128-node
      block, bf16), one-hot PE scatter -> T_in[own nodes], finalized
      (x inv_cnt, @Win^T) and kept RESIDENT in SBUF (bf16, [n,d] blocks).
      No collective needed for T_in.
  A2: same for src-grouped edges -> T_out[own], finalized, transposed
      to [d,n] fp32 and written per-block to DRAM; one AllGather makes
      the full [d, 50176] table; relayout DMAs stage one half
      ([128, 25088] fp32, 98KB/partition) in SBUF at a time.
  C:  edges sorted by (dst_half, src_block, dst), padded per
      (half, block) to 128-multiples so the tile->block map is uniform
      across cores (SPMD).  Per 1536-edge supertile: one gpsimd
      ap_gather (SBUF->SBUF, 8 Q7 cores, no DMA descriptors) expands
      T_out^T columns; per 128-edge tile: ones-bcast matmul + is_equal
      builds the transposed one-hot, then 2 accumulated matmuls give
      psum[d,e] = Wself^T-mm(xT) + T_in-block-mm(ohT); DVE adds the
      gathered T_out^T; ACT applies Prelu(+bias).  y is written bf16,
      transposed [d, e]; the host de-permutes.
"""

import sys

sys.path.insert(0, "/opt/trn_rl_repo")

import numpy as np
import ml_dtypes

import concourse.bacc as bacc
import concourse.bass as bass
import concourse.mybir as mybir
import concourse.tile as tile
from concourse import library_config
from concourse.bass_utils import run_bass_kernel_spmd
from concourse.masks import make_identity

BF = ml_dtypes.bfloat16

P = 128
D = 128
C = 8
E = 600000
N = 50000
NPC = N // C            # 6250
NB = (NPC + P - 1) // P  # 49
NBP = NB * P            # 6272
TROWS = C * NBP         # 50176
HALFT = TROWS // 2      # 25088
KC = 12
SUP = P * KC            # 1536

F32 = mybir.dt.float32
BF16 = mybir.dt.bfloat16
I16 = mybir.dt.int16

PRELU = mybir.ActivationFunctionType.Prelu


def build_kernel(KA_dst, KA_src, NSUP_H, BLK):
    """NSUP_H: (nsup_half0, nsup_half1). BLK: tuple of per-tile block ids,
    len = 12*(nsup0+nsup1), uniform across cores."""
    NSUP = NSUP_H[0] + NSUP_H[1]
    SIDX = SUP // 16

    nc = bacc.Bacc(None, target_bir_lowering=False, debug=False)

    # ---- I/O ----
    agat_d = nc.dram_tensor("agat_d", [NB, P, KA_dst * D], BF16, kind="ExternalInput")
    va_d = nc.dram_tensor("va_d", [NB, P, KA_dst], F32, kind="ExternalInput")
    agat_s = nc.dram_tensor("agat_s", [NB, P, KA_src * D], BF16, kind="ExternalInput")
    va_s = nc.dram_tensor("va_s", [NB, P, KA_src], F32, kind="ExternalInput")
    invc_d = nc.dram_tensor("invc_d", [P, NB], F32, kind="ExternalInput")
    invc_s = nc.dram_tensor("invc_s", [P, NB], F32, kind="ExternalInput")
    xT_d = nc.dram_tensor("xT", [NSUP, P, SUP], BF16, kind="ExternalInput")
    srcv_d = nc.dram_tensor("srcv", [NSUP, 1, SUP], BF16, kind="ExternalInput")
    gidx_d = nc.dram_tensor("gidx", [NSUP, P, SIDX], I16, kind="ExternalInput")
    win = nc.dram_tensor("win", [D, D], BF16, kind="ExternalInput")      # W_in_w.T
    wout = nc.dram_tensor("wout", [D, D], BF16, kind="ExternalInput")    # W_out_w.T
    wselfT = nc.dram_tensor("wselfT", [D, D], BF16, kind="ExternalInput")  # W_self_w.T
    bbcol = nc.dram_tensor("bbcol", [P, 1], F32, kind="ExternalInput")   # W_self_b col
    iota_in = nc.dram_tensor("iota", [P, P], F32, kind="ExternalInput")
    iotac_in = nc.dram_tensor("iotac", [P, 1], F32, kind="ExternalInput")
    y = nc.dram_tensor("y", [NSUP * P, SUP], BF16, kind="ExternalOutput")

    with tile.TileContext(nc) as tc:
        with (
            tc.tile_pool(name="const", bufs=1) as cpool,
            tc.tile_pool(name="sbuf", bufs=3) as pool,
            tc.tile_pool(name="small", bufs=4) as spool,
            tc.tile_pool(name="psumA", bufs=1, space="PSUM") as psum,
            tc.tile_pool(name="psumC", bufs=2, space="PSUM") as psumc,
            tc.tile_pool(name="dram", bufs=1, space="DRAM") as dram,
        ):
            nc.gpsimd.load_library(library_config.mlp)
            # ---- constants ----
            ident = cpool.tile([P, P], BF16)
            make_identity(nc, ident[:])
            iota_t = cpool.tile([P, P], F32)
            nc.sync.dma_start(out=iota_t[:], in_=iota_in[:])
            iota_c = cpool.tile([P, 1], F32)
            nc.sync.dma_start(out=iota_c[:], in_=iotac_in[:])
            ones_t = cpool.tile([1, P], BF16)
            nc.vector.memset(ones_t[:], 1.0)
            win_t = cpool.tile([D, D], BF16)
            nc.sync.dma_start(out=win_t[:], in_=win[:])
            wout_t = cpool.tile([D, D], BF16)
            nc.sync.dma_start(out=wout_t[:], in_=wout[:])
            wselfT_t = cpool.tile([D, D], BF16)
            nc.sync.dma_start(out=wselfT_t[:], in_=wselfT[:])
            bb_t = cpool.tile([P, 1], F32)
            nc.sync.dma_start(out=bb_t[:], in_=bbcol[:])
            invc_d_t = cpool.tile([P, NB], F32)
            nc.sync.dma_start(out=invc_d_t[:], in_=invc_d[:])
            invc_s_t = cpool.tile([P, NB], F32)
            nc.sync.dma_start(out=invc_s_t[:], in_=invc_s[:])
            # resident T_in blocks [n_local, dout] bf16, block b at cols b*128
            tinb = cpool.tile([P, NB * D], BF16)
            cc_in = dram.tile([NBP, D], BF16)
            cc_out = dram.tile([TROWS, D], BF16)

            # ---- phase A ----
            def phase_a(agat, va, KA, invc_t, w_t, to_tin):
                for b in range(NB):
                    valt = spool.tile([P, KA], F32, tag="aval")
                    nc.sync.dma_start(out=valt[:], in_=va[b])
                    gat = pool.tile([P, KA * D], BF16, tag="agather")
                    nc.sync.dma_start(out=gat[:], in_=agat[b])
                    ps = psum.tile([P, D], F32, tag="pA")
                    for j in range(KA):
                        oh = spool.tile([P, P], BF16, tag="oh")
                        nc.vector.tensor_scalar(
                            oh[:], iota_t[:], valt[:, j : j + 1], None,
                            mybir.AluOpType.is_equal,
                        )
                        nc.tensor.matmul(
                            ps[:], oh[:], gat[:, j * D : (j + 1) * D],
                            start=(j == 0), stop=(j == KA - 1),
                        )
                    means = spool.tile([P, D], BF16, tag="means")
                    nc.vector.tensor_scalar(
                        means[:], ps[:], invc_t[:, b : b + 1], None,
                        mybir.AluOpType.mult,
                    )
                    pst = psum.tile([P, D], BF16, tag="pB")
                    nc.tensor.transpose(pst[:], means[:], ident[:])
                    meansT = spool.tile([P, D], BF16, tag="meansT")
                    nc.scalar.copy(out=meansT[:], in_=pst[:])
                    psT = psum.tile([P, D], F32, tag="pC")
                    nc.tensor.matmul(psT[:], meansT[:], w_t[:], start=True, stop=True)
                    if to_tin:
                        # [node, dout] -> resident bf16 block
                        nc.scalar.copy(out=tinb[:, b * D : (b + 1) * D], in_=psT[:])
                    else:
                        # [node, dout] bf16 rows staged to DRAM for AllGather
                        tt = spool.tile([P, D], BF16, tag="tt")
                        nc.scalar.copy(out=tt[:], in_=psT[:])
                        nc.sync.dma_start(
                            out=cc_in[b * P : (b + 1) * P, :], in_=tt[:]
                        )

            phase_a(agat_s, va_s, KA_src, invc_s_t, wout_t, False)
            nc.gpsimd.collective_compute(
                "AllGather", mybir.AluOpType.bypass,
                replica_groups=[list(range(C))],
                ins=[cc_in.opt()], outs=[cc_out.opt()],
            )
            phase_a(agat_d, va_d, KA_dst, invc_d_t, win_t, True)
            tc.strict_bb_all_engine_barrier()

            # ---- phase C ----
            s_global = 0
            for h in range(2):
                tbl_half = cc_out[h * HALFT : (h + 1) * HALFT, :]
                for _ in range(NSUP_H[h]):
                    s = s_global
                    s_global += 1
                    xT_t = pool.tile([P, SUP], BF16, tag="xT")
                    nc.sync.dma_start(out=xT_t[:], in_=xT_d[s])
                    srcv_t = spool.tile([1, SUP], BF16, tag="srcv")
                    nc.sync.dma_start(out=srcv_t[:], in_=srcv_d[s])
                    gix = spool.tile([P, SIDX], I16, tag="gix")
                    nc.sync.dma_start(out=gix[:], in_=gidx_d[s])
                    go = pool.tile([P, SUP], BF16, tag="go")
                    nc.gpsimd.dma_gather(
                        out_ap=go[:].rearrange("p (a e) -> p a e", a=1),
                        in_ap=tbl_half,
                        idxs_ap=gix[:],
                        num_idxs=SUP, num_idxs_reg=SUP, elem_size=D,
                        transpose=True, single_packet=False,
                    )
                    yo = pool.tile([P, SUP], BF16, tag="yo")
                    W = 4 * D  # 512-wide groups
                    for g in range(KC // 4):
                        slg = slice(g * W, (g + 1) * W)
                        psB = psumc.tile([P, W], F32, tag="qB")
                        nc.tensor.matmul(
                            psB[:], ones_t[:], srcv_t[:, slg], start=True, stop=True
                        )
                        ohT = spool.tile([P, W], BF16, tag="ohT")
                        nc.vector.tensor_scalar(
                            ohT[:], psB[:], iota_c[:, 0:1], None,
                            mybir.AluOpType.is_equal,
                        )
                        psA = psumc.tile([P, W], F32, tag="qA")
                        nc.tensor.matmul(
                            psA[:], wselfT_t[:], xT_t[:, slg], start=True, stop=False
                        )
                        for k in range(4):
                            t = g * 4 + k
                            blk = BLK[s * KC + t]
                            sk = slice(k * D, (k + 1) * D)
                            nc.tensor.matmul(
                                psA[:, sk], tinb[:, blk * D : (blk + 1) * D],
                                ohT[:, sk], start=False, stop=True,
                                skip_group_check=True,
                            )
                        st = spool.tile([P, W], BF16, tag="st")
                        nc.vector.tensor_add(st[:], psA[:], go[:, slg])
                        nc.scalar.activation(
                            yo[:, slg], st[:], PRELU,
                            bias=bb_t[:, 0:1], scale=1.0, alpha=0.2,
                        )
                    nc.sync.dma_start(out=y[s * P : (s + 1) * P, :], in_=yo[:])

    nc.compile()
    return nc


def prepare_inputs(edge_attr, edge_index, W_self_w, W_self_b, W_in_w, W_out_w):
    edge_attr = np.ascontiguousarray(edge_attr, dtype=np.float32)
    src = np.asarray(edge_index[0], dtype=np.int64)
    dst = np.asarray(edge_index[1], dtype=np.int64)

    win = np.ascontiguousarray(np.asarray(W_in_w, np.float32).T).astype(BF)
    wout = np.ascontiguousarray(np.asarray(W_out_w, np.float32).T).astype(BF)
    wselfT = np.ascontiguousarray(np.asarray(W_self_w, np.float32).T).astype(BF)
    bbcol = np.asarray(W_self_b, np.float32).reshape(P, 1)
    iota = np.tile(np.arange(P, dtype=np.float32)[None, :], (P, 1))
    iotac = np.arange(P, dtype=np.float32).reshape(P, 1)

    # ---- phase A dense buckets (same scheme as v1) ----
    def build_a(node_of_edge):
        core = node_of_edge // NPC
        local = node_of_edge - core * NPC
        inblk = (local & 127).astype(np.float32)
        key = (core * NB + (local >> 7)).astype(np.int64)
        order = np.argsort(key, kind="stable")
        cnts = np.bincount(key, minlength=C * NB)
        KA = max(1, int(np.ceil(cnts.max() / P)))
        starts = np.zeros(C * NB, dtype=np.int64)
        np.cumsum(cnts[:-1], out=starts[1:])
        pos = np.arange(E, dtype=np.int64) - starts[key[order]]
        slot = key[order] * (P * KA) + pos
        agat = np.zeros((C * NB * P * KA, D), dtype=BF)
        agat[slot] = edge_attr[order].astype(BF)
        agat = agat.reshape(C, NB, P, KA * D)
        va = np.full((C * NB * P * KA), -1.0, dtype=np.float32)
        va[slot] = inblk[order]
        va = va.reshape(C, NB, P, KA)
        cnt_node = np.bincount(node_of_edge, minlength=N).astype(np.float32)
        inv = 1.0 / np.maximum(cnt_node, 1.0)
        inv_pad = np.zeros((C, NBP), dtype=np.float32)
        inv_pad[:, :NPC] = inv.reshape(C, NPC)
        invc = np.ascontiguousarray(inv_pad.reshape(C, NB, P).transpose(0, 2, 1))
        return KA, agat, va, invc

    KA_dst, agat_d, va_d, invc_d = build_a(dst)
    KA_src, agat_s, va_s, invc_s = build_a(src)

    # ---- phase C structure (uniform across cores) ----
    trow = lambda n: (n // NPC) * NBP + (n % NPC)
    rows_d = trow(dst)
    core_e = src // NPC
    src_loc = src - core_e * NPC
    blk_e = src_loc >> 7
    half_e = (rows_d >= HALFT).astype(np.int64)

    # per-core, per (half, block) counts -> uniform tile counts
    cnt = np.zeros((C, 2, NB), dtype=np.int64)
    for c in range(C):
        m = core_e == c
        np.add.at(cnt[c], (half_e[m], blk_e[m]), 1)
    maxcnt = cnt.max(axis=0)  # [2, NB]
    T_hb = np.maximum(1, np.ceil(maxcnt / P).astype(np.int64))  # tiles per (h,b)
    tiles_h = T_hb.sum(axis=1)
    NSUP_H = tuple(int(np.ceil(th / KC)) for th in tiles_h)
    NSUP = NSUP_H[0] + NSUP_H[1]
    NT = NSUP * KC
    # per-tile block ids (uniform): concat [b]*T_hb then pad to supertile mult
    BLK = []
    tile_base = np.zeros((2, NB), dtype=np.int64)  # first tile of (h,b)
    tb = 0
    for h in range(2):
        for b in range(NB):
            tile_base[h, b] = tb
            BLK.extend([b] * int(T_hb[h, b]))
            tb += int(T_hb[h, b])
        pad = NSUP_H[h] * KC - (tb - (0 if h == 0 else NSUP_H[0] * KC))
        BLK.extend([0] * pad)
        tb += pad
    BLK = tuple(BLK)
    assert len(BLK) == NT

    def wrap_idx(vals):
        S = len(vals) // 16
        t = np.zeros((16, S), dtype=np.int16)
        t[np.arange(len(vals)) % 16, np.arange(len(vals)) // 16] = vals.astype(
            np.int16
        )
        return np.tile(t, (8, 1))

    in_maps = []
    posts = []
    for c in range(C):
        m = core_e == c
        eids = np.nonzero(m)[0]
        # sort by (half, block, dst)
        okey = (half_e[eids] * NB + blk_e[eids]) * (2 * N) + dst[eids]
        order = np.argsort(okey, kind="stable")
        eids = eids[order]
        # slot assignment: per (h,b) segment starts at tile_base*P
        slot_edge = np.full(NT * P, -1, dtype=np.int64)
        pos = 0
        for h in range(2):
            for b in range(NB):
                k = int(cnt[c, h, b])
                base = int(tile_base[h, b]) * P
                slot_edge[base : base + k] = eids[pos : pos + k]
                pos += k
        assert pos == len(eids)
        valid = slot_edge >= 0
        ge = np.where(valid, slot_edge, 0)
        xs = np.where(valid[:, None], edge_attr[ge], 0).astype(BF)
        # [NT*P, D] -> [NSUP, P(d), SUP(cols)]
        xT = np.ascontiguousarray(
            xs.reshape(NSUP, SUP, D).transpose(0, 2, 1)
        )
        sv = np.where(valid, src_loc[ge] & 127, -1).astype(np.float32).astype(BF)
        sv = np.ascontiguousarray(sv.reshape(NSUP, 1, SUP))
        gi = np.where(valid, rows_d[ge] - half_e[ge] * HALFT, 0)
        gidx = np.stack(
            [wrap_idx(gi[s * SUP : (s + 1) * SUP]) for s in range(NSUP)]
        )
        in_maps.append(
            dict(
                agat_d=agat_d[c], va_d=va_d[c], agat_s=agat_s[c], va_s=va_s[c],
                invc_d=invc_d[c], invc_s=invc_s[c],
                xT=xT, srcv=sv, gidx=gidx,
                win=win, wout=wout, wselfT=wselfT, bbcol=bbcol,
                iota=iota, iotac=iotac,
            )
        )
        posts.append((slot_edge, valid))

    def postprocess(results):
        full = np.empty((E, D), dtype=np.float32)
        for c in range(C):
            slot_edge, valid = posts[c]
            yv = np.asarray(results[c]["y"]).astype(np.float32)
            yv = yv.reshape(NSUP, P, SUP).transpose(0, 2, 1).reshape(NT * P, D)
            full[slot_edge[valid]] = yv[valid]
        return full

    params = (KA_dst, KA_src, NSUP_H, BLK)
    return params, in_maps, postprocess


_NC_CACHE = {}


def run(inputs, trace=False, trace_kwargs=None):
    params, in_maps, post = prepare_inputs(
        inputs["edge_attr"],
        inputs["edge_index"],
        inputs["W_self_w"],
        inputs["W_self_b"],
        inputs["W_in_w"],
        inputs["W_out_w"],
    )
    key = params
    if key not in _NC_CACHE:
        _NC_CACHE[key] = build_kernel(*params)
    nc = _NC_CACHE[key]
    kw = {}
    if trace:
        kw["trace"] = True
        if trace_kwargs:
            kw.update(trace_kwargs)
    res = run_bass_kernel_spmd(nc, in_maps, core_ids=list(range(C)), **kw)
    return post(res.results), res


def kernel(**inputs) -> np.ndarray:
    out, _ = run(inputs)
    return out.astype(np.float32)


# revision 3
# speedup vs baseline: 1.1215x; 1.0296x over previous
"""DirectedEdgeConv Trainium2 kernel, 8-core SPMD — v2 (descriptor-free).

out[e] = prelu(x[e] @ Wself^T + b + T_in[src[e]] + T_out[dst[e]], 0.2)
  T_in  = scatter_mean(x, dst) @ Win^T    (bias folded into prelu's bias)
  T_out = scatter_mean(x, src) @ Wout^T

Sharding: edges assigned to core own(src). Per core:
  A1: stream dst-owner-grouped edges (dense KA buckets per 128-node
      block, bf16), one-hot PE scatter -> T_in[own nodes], finalized
      (x inv_cnt, @Win^T) and kept RESIDENT in SBUF (bf16, [n,d] blocks).
      No collective needed for T_in.
  A2: same for src-grouped edges -> T_out[own], finalized, transposed
      to [d,n] fp32 and written per-block to DRAM; one AllGather makes
      the full [d, 50176] table; relayout DMAs stage one half
      ([128, 25088] fp32, 98KB/partition) in SBUF at a time.
  C:  edges sorted by (dst_half, src_block, dst), padded per
      (half, block) to 128-multiples so the tile->block map is uniform
      across cores (SPMD).  Per 1536-edge supertile: one gpsimd
      ap_gather (SBUF->SBUF, 8 Q7 cores, no DMA descriptors) expands
      T_out^T columns; per 128-edge tile: ones-bcast matmul + is_equal
      builds the transposed one-hot, then 2 accumulated matmuls give
      psum[d,e] = Wself^T-mm(xT) + T_in-block-mm(ohT); DVE adds the
      gathered T_out^T; ACT applies Prelu(+bias).  y is written bf16,
      transposed [d, e]; the host de-permutes.
"""

import sys

sys.path.insert(0, "/opt/trn_rl_repo")

import numpy as np
import ml_dtypes

import concourse.bacc as bacc
import concourse.bass as bass
import concourse.mybir as mybir
import concourse.tile as tile
from concourse import library_config
from concourse.bass_utils import run_bass_kernel_spmd
from concourse.masks import make_identity

BF = ml_dtypes.bfloat16

P = 128
D = 128
C = 8
E = 600000
N = 50000
NPC = N // C            # 6250
NB = (NPC + P - 1) // P  # 49
NBP = NB * P            # 6272
TROWS = C * NBP         # 50176
HALFT = TROWS // 2      # 25088
KC = 24
SUP = P * KC            # 3072

F32 = mybir.dt.float32
BF16 = mybir.dt.bfloat16
I16 = mybir.dt.int16

PRELU = mybir.ActivationFunctionType.Prelu


def build_kernel(KA_dst, KA_src, NSUP_H, BLK):
    """NSUP_H: (nsup_half0, nsup_half1). BLK: tuple of per-tile block ids,
    len = 12*(nsup0+nsup1), uniform across cores."""
    NSUP = NSUP_H[0] + NSUP_H[1]
    SIDX = SUP // 16

    nc = bacc.Bacc(None, target_bir_lowering=False, debug=False)

    # ---- I/O ----
    agat_d = nc.dram_tensor("agat_d", [NB, P, KA_dst * D], BF16, kind="ExternalInput")
    va_d = nc.dram_tensor("va_d", [NB, P, KA_dst], F32, kind="ExternalInput")
    agat_s = nc.dram_tensor("agat_s", [NB, P, KA_src * D], BF16, kind="ExternalInput")
    va_s = nc.dram_tensor("va_s", [NB, P, KA_src], F32, kind="ExternalInput")
    invc_d = nc.dram_tensor("invc_d", [P, NB], F32, kind="ExternalInput")
    invc_s = nc.dram_tensor("invc_s", [P, NB], F32, kind="ExternalInput")
    xT_d = nc.dram_tensor("xT", [NSUP, P, SUP], BF16, kind="ExternalInput")
    srcv_d = nc.dram_tensor("srcv", [NSUP, 1, SUP], BF16, kind="ExternalInput")
    gidx_d = nc.dram_tensor("gidx", [NSUP, P, SIDX], I16, kind="ExternalInput")
    win = nc.dram_tensor("win", [D, D], BF16, kind="ExternalInput")      # W_in_w.T
    wout = nc.dram_tensor("wout", [D, D], BF16, kind="ExternalInput")    # W_out_w.T
    wselfT = nc.dram_tensor("wselfT", [D, D], BF16, kind="ExternalInput")  # W_self_w.T
    bbcol = nc.dram_tensor("bbcol", [P, 1], F32, kind="ExternalInput")   # W_self_b col
    iota_in = nc.dram_tensor("iota", [P, P], F32, kind="ExternalInput")
    iotac_in = nc.dram_tensor("iotac", [P, 1], F32, kind="ExternalInput")
    y = nc.dram_tensor("y", [NSUP * P, SUP], BF16, kind="ExternalOutput")

    with tile.TileContext(nc) as tc:
        with (
            tc.tile_pool(name="const", bufs=1) as cpool,
            tc.tile_pool(name="sbuf", bufs=3) as pool,
            tc.tile_pool(name="small", bufs=4) as spool,
            tc.tile_pool(name="psumA", bufs=1, space="PSUM") as psum,
            tc.tile_pool(name="psumC", bufs=2, space="PSUM") as psumc,
            tc.tile_pool(name="dram", bufs=1, space="DRAM") as dram,
        ):
            nc.gpsimd.load_library(library_config.mlp)
            # ---- constants ----
            ident = cpool.tile([P, P], BF16)
            make_identity(nc, ident[:])
            iota_t = cpool.tile([P, P], F32)
            nc.sync.dma_start(out=iota_t[:], in_=iota_in[:])
            iota_c = cpool.tile([P, 1], F32)
            nc.sync.dma_start(out=iota_c[:], in_=iotac_in[:])
            ones_t = cpool.tile([1, P], BF16)
            nc.vector.memset(ones_t[:], 1.0)
            win_t = cpool.tile([D, D], BF16)
            nc.sync.dma_start(out=win_t[:], in_=win[:])
            wout_t = cpool.tile([D, D], BF16)
            nc.sync.dma_start(out=wout_t[:], in_=wout[:])
            wselfT_t = cpool.tile([D, D], BF16)
            nc.sync.dma_start(out=wselfT_t[:], in_=wselfT[:])
            bb_t = cpool.tile([P, 1], F32)
            nc.sync.dma_start(out=bb_t[:], in_=bbcol[:])
            invc_d_t = cpool.tile([P, NB], F32)
            nc.sync.dma_start(out=invc_d_t[:], in_=invc_d[:])
            invc_s_t = cpool.tile([P, NB], F32)
            nc.sync.dma_start(out=invc_s_t[:], in_=invc_s[:])
            # resident T_in blocks [n_local, dout] bf16, one tile per block
            tinb = [cpool.tile([P, D], BF16, name=f"tinb{b}") for b in range(NB)]
            cc_in = dram.tile([NBP, D], BF16)
            cc_out = dram.tile([TROWS, D], BF16)

            # ---- phase A ----
            def phase_a(agat, va, KA, invc_t, w_t, to_tin):
                for b in range(NB):
                    valt = spool.tile([P, KA], F32, tag="aval")
                    nc.sync.dma_start(out=valt[:], in_=va[b])
                    gat = pool.tile([P, KA * D], BF16, tag="agather")
                    nc.sync.dma_start(out=gat[:], in_=agat[b])
                    ps = psum.tile([P, D], F32, tag="pA")
                    for j in range(KA):
                        oh = spool.tile([P, P], BF16, tag="oh")
                        nc.vector.tensor_scalar(
                            oh[:], iota_t[:], valt[:, j : j + 1], None,
                            mybir.AluOpType.is_equal,
                        )
                        nc.tensor.matmul(
                            ps[:], oh[:], gat[:, j * D : (j + 1) * D],
                            start=(j == 0), stop=(j == KA - 1),
                        )
                    means = spool.tile([P, D], BF16, tag="means")
                    nc.vector.tensor_scalar(
                        means[:], ps[:], invc_t[:, b : b + 1], None,
                        mybir.AluOpType.mult,
                    )
                    pst = psum.tile([P, D], BF16, tag="pB")
                    nc.tensor.transpose(pst[:], means[:], ident[:])
                    meansT = spool.tile([P, D], BF16, tag="meansT")
                    nc.scalar.copy(out=meansT[:], in_=pst[:])
                    psT = psum.tile([P, D], F32, tag="pC")
                    nc.tensor.matmul(psT[:], meansT[:], w_t[:], start=True, stop=True)
                    if to_tin:
                        # [node, dout] -> resident bf16 block
                        nc.scalar.copy(out=tinb[b][:], in_=psT[:])
                    else:
                        # [node, dout] bf16 rows staged to DRAM for AllGather
                        tt = spool.tile([P, D], BF16, tag="tt")
                        nc.scalar.copy(out=tt[:], in_=psT[:])
                        nc.sync.dma_start(
                            out=cc_in[b * P : (b + 1) * P, :], in_=tt[:]
                        )

            phase_a(agat_s, va_s, KA_src, invc_s_t, wout_t, False)
            nc.gpsimd.collective_compute(
                "AllGather", mybir.AluOpType.bypass,
                replica_groups=[list(range(C))],
                ins=[cc_in.opt()], outs=[cc_out.opt()],
            )
            phase_a(agat_d, va_d, KA_dst, invc_d_t, win_t, True)

            # ---- phase C ----
            s_global = 0
            for h in range(2):
                tbl_half = cc_out[h * HALFT : (h + 1) * HALFT, :]
                for _ in range(NSUP_H[h]):
                    s = s_global
                    s_global += 1
                    xT_t = pool.tile([P, SUP], BF16, tag="xT")
                    nc.sync.dma_start(out=xT_t[:], in_=xT_d[s])
                    srcv_t = spool.tile([1, SUP], BF16, tag="srcv")
                    nc.sync.dma_start(out=srcv_t[:], in_=srcv_d[s])
                    gix = spool.tile([P, SIDX], I16, tag="gix")
                    nc.sync.dma_start(out=gix[:], in_=gidx_d[s])
                    go = pool.tile([P, SUP], BF16, tag="go")
                    nc.gpsimd.dma_gather(
                        out_ap=go[:].rearrange("p (a e) -> p a e", a=1),
                        in_ap=tbl_half,
                        idxs_ap=gix[:],
                        num_idxs=SUP, num_idxs_reg=SUP, elem_size=D,
                        transpose=True, single_packet=False,
                    )
                    yo = pool.tile([P, SUP], BF16, tag="yo")
                    W = 4 * D  # 512-wide groups
                    for g in range(KC // 4):
                        slg = slice(g * W, (g + 1) * W)
                        psB = psumc.tile([P, W], F32, tag="qB")
                        nc.tensor.matmul(
                            psB[:], ones_t[:], srcv_t[:, slg], start=True, stop=True
                        )
                        ohT = spool.tile([P, W], BF16, tag="ohT")
                        nc.vector.tensor_scalar(
                            ohT[:], psB[:], iota_c[:, 0:1], None,
                            mybir.AluOpType.is_equal,
                        )
                        psA = psumc.tile([P, W], F32, tag="qA")
                        nc.tensor.matmul(
                            psA[:], wselfT_t[:], xT_t[:, slg], start=True, stop=False
                        )
                        for k in range(4):
                            t = g * 4 + k
                            blk = BLK[s * KC + t]
                            sk = slice(k * D, (k + 1) * D)
                            nc.tensor.matmul(
                                psA[:, sk], tinb[blk][:],
                                ohT[:, sk], start=False, stop=True,
                                skip_group_check=True,
                            )
                        st = spool.tile([P, W], BF16, tag="st")
                        nc.vector.tensor_add(st[:], psA[:], go[:, slg])
                        nc.scalar.activation(
                            yo[:, slg], st[:], PRELU,
                            bias=bb_t[:, 0:1], scale=1.0, alpha=0.2,
                        )
                    nc.sync.dma_start(out=y[s * P : (s + 1) * P, :], in_=yo[:])

    nc.compile()
    return nc


def prepare_inputs(edge_attr, edge_index, W_self_w, W_self_b, W_in_w, W_out_w):
    edge_attr = np.ascontiguousarray(edge_attr, dtype=np.float32)
    src = np.asarray(edge_index[0], dtype=np.int64)
    dst = np.asarray(edge_index[1], dtype=np.int64)

    win = np.ascontiguousarray(np.asarray(W_in_w, np.float32).T).astype(BF)
    wout = np.ascontiguousarray(np.asarray(W_out_w, np.float32).T).astype(BF)
    wselfT = np.ascontiguousarray(np.asarray(W_self_w, np.float32).T).astype(BF)
    bbcol = np.asarray(W_self_b, np.float32).reshape(P, 1)
    iota = np.tile(np.arange(P, dtype=np.float32)[None, :], (P, 1))
    iotac = np.arange(P, dtype=np.float32).reshape(P, 1)

    # ---- phase A dense buckets (same scheme as v1) ----
    def build_a(node_of_edge):
        core = node_of_edge // NPC
        local = node_of_edge - core * NPC
        inblk = (local & 127).astype(np.float32)
        key = (core * NB + (local >> 7)).astype(np.int64)
        order = np.argsort(key, kind="stable")
        cnts = np.bincount(key, minlength=C * NB)
        KA = max(1, int(np.ceil(cnts.max() / P)))
        starts = np.zeros(C * NB, dtype=np.int64)
        np.cumsum(cnts[:-1], out=starts[1:])
        pos = np.arange(E, dtype=np.int64) - starts[key[order]]
        slot = key[order] * (P * KA) + pos
        agat = np.zeros((C * NB * P * KA, D), dtype=BF)
        agat[slot] = edge_attr[order].astype(BF)
        agat = agat.reshape(C, NB, P, KA * D)
        va = np.full((C * NB * P * KA), -1.0, dtype=np.float32)
        va[slot] = inblk[order]
        va = va.reshape(C, NB, P, KA)
        cnt_node = np.bincount(node_of_edge, minlength=N).astype(np.float32)
        inv = 1.0 / np.maximum(cnt_node, 1.0)
        inv_pad = np.zeros((C, NBP), dtype=np.float32)
        inv_pad[:, :NPC] = inv.reshape(C, NPC)
        invc = np.ascontiguousarray(inv_pad.reshape(C, NB, P).transpose(0, 2, 1))
        return KA, agat, va, invc

    KA_dst, agat_d, va_d, invc_d = build_a(dst)
    KA_src, agat_s, va_s, invc_s = build_a(src)

    # ---- phase C structure (uniform across cores) ----
    trow = lambda n: (n // NPC) * NBP + (n % NPC)
    rows_d = trow(dst)
    core_e = src // NPC
    src_loc = src - core_e * NPC
    blk_e = src_loc >> 7
    half_e = (rows_d >= HALFT).astype(np.int64)

    # per-core, per (half, block) counts -> uniform tile counts
    cnt = np.zeros((C, 2, NB), dtype=np.int64)
    for c in range(C):
        m = core_e == c
        np.add.at(cnt[c], (half_e[m], blk_e[m]), 1)
    maxcnt = cnt.max(axis=0)  # [2, NB]
    T_hb = np.maximum(1, np.ceil(maxcnt / P).astype(np.int64))  # tiles per (h,b)
    tiles_h = T_hb.sum(axis=1)
    NSUP_H = tuple(int(np.ceil(th / KC)) for th in tiles_h)
    NSUP = NSUP_H[0] + NSUP_H[1]
    NT = NSUP * KC
    # per-tile block ids (uniform): concat [b]*T_hb then pad to supertile mult
    BLK = []
    tile_base = np.zeros((2, NB), dtype=np.int64)  # first tile of (h,b)
    tb = 0
    for h in range(2):
        for b in range(NB):
            tile_base[h, b] = tb
            BLK.extend([b] * int(T_hb[h, b]))
            tb += int(T_hb[h, b])
        pad = NSUP_H[h] * KC - (tb - (0 if h == 0 else NSUP_H[0] * KC))
        BLK.extend([0] * pad)
        tb += pad
    BLK = tuple(BLK)
    assert len(BLK) == NT

    def wrap_idx(vals):
        S = len(vals) // 16
        t = np.zeros((16, S), dtype=np.int16)
        t[np.arange(len(vals)) % 16, np.arange(len(vals)) // 16] = vals.astype(
            np.int16
        )
        return np.tile(t, (8, 1))

    in_maps = []
    posts = []
    for c in range(C):
        m = core_e == c
        eids = np.nonzero(m)[0]
        # sort by (half, block, dst)
        okey = (half_e[eids] * NB + blk_e[eids]) * (2 * N) + dst[eids]
        order = np.argsort(okey, kind="stable")
        eids = eids[order]
        # slot assignment: per (h,b) segment starts at tile_base*P
        slot_edge = np.full(NT * P, -1, dtype=np.int64)
        pos = 0
        for h in range(2):
            for b in range(NB):
                k = int(cnt[c, h, b])
                base = int(tile_base[h, b]) * P
                slot_edge[base : base + k] = eids[pos : pos + k]
                pos += k
        assert pos == len(eids)
        valid = slot_edge >= 0
        ge = np.where(valid, slot_edge, 0)
        xs = np.where(valid[:, None], edge_attr[ge], 0).astype(BF)
        # [NT*P, D] -> [NSUP, P(d), SUP(cols)]
        xT = np.ascontiguousarray(
            xs.reshape(NSUP, SUP, D).transpose(0, 2, 1)
        )
        sv = np.where(valid, src_loc[ge] & 127, -1).astype(np.float32).astype(BF)
        sv = np.ascontiguousarray(sv.reshape(NSUP, 1, SUP))
        gi = np.where(valid, rows_d[ge] - half_e[ge] * HALFT, 0)
        gidx = np.stack(
            [wrap_idx(gi[s * SUP : (s + 1) * SUP]) for s in range(NSUP)]
        )
        in_maps.append(
            dict(
                agat_d=agat_d[c], va_d=va_d[c], agat_s=agat_s[c], va_s=va_s[c],
                invc_d=invc_d[c], invc_s=invc_s[c],
                xT=xT, srcv=sv, gidx=gidx,
                win=win, wout=wout, wselfT=wselfT, bbcol=bbcol,
                iota=iota, iotac=iotac,
            )
        )
        posts.append((slot_edge, valid))

    def postprocess(results):
        full = np.empty((E, D), dtype=np.float32)
        for c in range(C):
            slot_edge, valid = posts[c]
            yv = np.asarray(results[c]["y"]).astype(np.float32)
            yv = yv.reshape(NSUP, P, SUP).transpose(0, 2, 1).reshape(NT * P, D)
            full[slot_edge[valid]] = yv[valid]
        return full

    params = (KA_dst, KA_src, NSUP_H, BLK)
    return params, in_maps, postprocess


_NC_CACHE = {}


def run(inputs, trace=False, trace_kwargs=None):
    params, in_maps, post = prepare_inputs(
        inputs["edge_attr"],
        inputs["edge_index"],
        inputs["W_self_w"],
        inputs["W_self_b"],
        inputs["W_in_w"],
        inputs["W_out_w"],
    )
    key = params
    if key not in _NC_CACHE:
        _NC_CACHE[key] = build_kernel(*params)
    nc = _NC_CACHE[key]
    kw = {}
    if trace:
        kw["trace"] = True
        if trace_kwargs:
            kw.update(trace_kwargs)
    res = run_bass_kernel_spmd(nc, in_maps, core_ids=list(range(C)), **kw)
    return post(res.results), res


def kernel(**inputs) -> np.ndarray:
    out, _ = run(inputs)
    return out.astype(np.float32)


# revision 4
# speedup vs baseline: 1.1778x; 1.0502x over previous
"""DirectedEdgeConv Trainium2 kernel, 8-core SPMD — v2 (descriptor-free).

out[e] = prelu(x[e] @ Wself^T + b + T_in[src[e]] + T_out[dst[e]], 0.2)
  T_in  = scatter_mean(x, dst) @ Win^T    (bias folded into prelu's bias)
  T_out = scatter_mean(x, src) @ Wout^T

Sharding: edges assigned to core own(src). Per core:
  A1: stream dst-owner-grouped edges (dense KA buckets per 128-node
      block, bf16), one-hot PE scatter -> T_in[own nodes], finalized
      (x inv_cnt, @Win^T) and kept RESIDENT in SBUF (bf16, [n,d] blocks).
      No collective needed for T_in.
  A2: same for src-grouped edges -> T_out[own], finalized, transposed
      to [d,n] fp32 and written per-block to DRAM; one AllGather makes
      the full [d, 50176] table; relayout DMAs stage one half
      ([128, 25088] fp32, 98KB/partition) in SBUF at a time.
  C:  edges sorted by (dst_half, src_block, dst), padded per
      (half, block) to 128-multiples so the tile->block map is uniform
      across cores (SPMD).  Per 1536-edge supertile: one gpsimd
      ap_gather (SBUF->SBUF, 8 Q7 cores, no DMA descriptors) expands
      T_out^T columns; per 128-edge tile: ones-bcast matmul + is_equal
      builds the transposed one-hot, then 2 accumulated matmuls give
      psum[d,e] = Wself^T-mm(xT) + T_in-block-mm(ohT); DVE adds the
      gathered T_out^T; ACT applies Prelu(+bias).  y is written bf16,
      transposed [d, e]; the host de-permutes.
"""

import sys

sys.path.insert(0, "/opt/trn_rl_repo")

import numpy as np
import ml_dtypes

import concourse.bacc as bacc
import concourse.bass as bass
import concourse.mybir as mybir
import concourse.tile as tile
from concourse import library_config
from concourse.bass_utils import run_bass_kernel_spmd
from concourse.masks import make_identity

BF = ml_dtypes.bfloat16

P = 128
D = 128
C = 8
E = 600000
N = 50000
NPC = N // C            # 6250
NB = (NPC + P - 1) // P  # 49
NBP = NB * P            # 6272
TROWS = C * NBP         # 50176
HALFT = TROWS // 2      # 25088
KC = 24
SUP = P * KC            # 3072

F32 = mybir.dt.float32
BF16 = mybir.dt.bfloat16
I16 = mybir.dt.int16

PRELU = mybir.ActivationFunctionType.Prelu


def build_kernel(KA_dst, KA_src, NSUP_H, BLK):
    """NSUP_H: (nsup_half0, nsup_half1). BLK: tuple of per-tile block ids,
    len = 12*(nsup0+nsup1), uniform across cores."""
    NSUP = NSUP_H[0] + NSUP_H[1]
    SIDX = SUP // 16

    nc = bacc.Bacc(None, target_bir_lowering=False, debug=False)

    # ---- I/O ----
    agat_d = nc.dram_tensor("agat_d", [NB, P, KA_dst * D], BF16, kind="ExternalInput")
    va_d = nc.dram_tensor("va_d", [NB, P, KA_dst], F32, kind="ExternalInput")
    agat_s = nc.dram_tensor("agat_s", [NB, P, KA_src * D], BF16, kind="ExternalInput")
    va_s = nc.dram_tensor("va_s", [NB, P, KA_src], F32, kind="ExternalInput")
    invc_d = nc.dram_tensor("invc_d", [P, NB], F32, kind="ExternalInput")
    invc_s = nc.dram_tensor("invc_s", [P, NB], F32, kind="ExternalInput")
    xT_d = nc.dram_tensor("xT", [NSUP, P, SUP], BF16, kind="ExternalInput")
    srcv_d = nc.dram_tensor("srcv", [NSUP, 1, SUP], BF16, kind="ExternalInput")
    gidx_d = nc.dram_tensor("gidx", [NSUP, P, SIDX], I16, kind="ExternalInput")
    win = nc.dram_tensor("win", [D, D], BF16, kind="ExternalInput")      # W_in_w.T
    wout = nc.dram_tensor("wout", [D, D], BF16, kind="ExternalInput")    # W_out_w.T
    wselfT = nc.dram_tensor("wselfT", [D, D], BF16, kind="ExternalInput")  # W_self_w.T
    bbcol = nc.dram_tensor("bbcol", [P, 1], F32, kind="ExternalInput")   # W_self_b col
    iota_in = nc.dram_tensor("iota", [P, P], F32, kind="ExternalInput")
    iotac_in = nc.dram_tensor("iotac", [P, 1], F32, kind="ExternalInput")
    y = nc.dram_tensor("y", [NSUP * P, SUP], BF16, kind="ExternalOutput")

    with tile.TileContext(nc) as tc:
        with (
            tc.tile_pool(name="const", bufs=1) as cpool,
            tc.tile_pool(name="sbuf", bufs=4) as pool,
            tc.tile_pool(name="small", bufs=4) as spool,
            tc.tile_pool(name="psumA", bufs=1, space="PSUM") as psum,
            tc.tile_pool(name="psumC", bufs=2, space="PSUM") as psumc,
            tc.tile_pool(name="dram", bufs=1, space="DRAM") as dram,
        ):
            nc.gpsimd.load_library(library_config.mlp)
            # ---- constants ----
            ident = cpool.tile([P, P], BF16)
            make_identity(nc, ident[:])
            iota_t = cpool.tile([P, P], F32)
            nc.sync.dma_start(out=iota_t[:], in_=iota_in[:])
            iota_c = cpool.tile([P, 1], F32)
            nc.sync.dma_start(out=iota_c[:], in_=iotac_in[:])
            ones_t = cpool.tile([1, P], BF16)
            nc.vector.memset(ones_t[:], 1.0)
            win_t = cpool.tile([D, D], BF16)
            nc.sync.dma_start(out=win_t[:], in_=win[:])
            wout_t = cpool.tile([D, D], BF16)
            nc.sync.dma_start(out=wout_t[:], in_=wout[:])
            wselfT_t = cpool.tile([D, D], BF16)
            nc.sync.dma_start(out=wselfT_t[:], in_=wselfT[:])
            bb_t = cpool.tile([P, 1], F32)
            nc.sync.dma_start(out=bb_t[:], in_=bbcol[:])
            invc_d_t = cpool.tile([P, NB], F32)
            nc.sync.dma_start(out=invc_d_t[:], in_=invc_d[:])
            invc_s_t = cpool.tile([P, NB], F32)
            nc.sync.dma_start(out=invc_s_t[:], in_=invc_s[:])
            # resident T_in blocks [n_local, dout] bf16, one tile per block
            tinb = [cpool.tile([P, D], BF16, name=f"tinb{b}") for b in range(NB)]
            cc_in = dram.tile([NBP, D], BF16)
            cc_out = dram.tile([TROWS, D], BF16)

            # ---- phase A ----
            def phase_a(agat, va, KA, invc_t, w_t, to_tin):
                for b in range(NB):
                    valt = spool.tile([P, KA], F32, tag="aval")
                    nc.sync.dma_start(out=valt[:], in_=va[b])
                    gat = pool.tile([P, KA * D], BF16, tag="agather")
                    nc.sync.dma_start(out=gat[:], in_=agat[b])
                    ps = psum.tile([P, D], F32, tag="pA")
                    for j in range(KA):
                        oh = spool.tile([P, P], BF16, tag="oh")
                        nc.vector.tensor_scalar(
                            oh[:], iota_t[:], valt[:, j : j + 1], None,
                            mybir.AluOpType.is_equal,
                        )
                        nc.tensor.matmul(
                            ps[:], oh[:], gat[:, j * D : (j + 1) * D],
                            start=(j == 0), stop=(j == KA - 1),
                        )
                    means = spool.tile([P, D], BF16, tag="means")
                    nc.vector.tensor_scalar(
                        means[:], ps[:], invc_t[:, b : b + 1], None,
                        mybir.AluOpType.mult,
                    )
                    pst = psum.tile([P, D], BF16, tag="pB")
                    nc.tensor.transpose(pst[:], means[:], ident[:])
                    meansT = spool.tile([P, D], BF16, tag="meansT")
                    nc.scalar.copy(out=meansT[:], in_=pst[:])
                    psT = psum.tile([P, D], F32, tag="pC")
                    nc.tensor.matmul(psT[:], meansT[:], w_t[:], start=True, stop=True)
                    if to_tin:
                        # [node, dout] -> resident bf16 block
                        nc.scalar.copy(out=tinb[b][:], in_=psT[:])
                    else:
                        # [node, dout] bf16 rows staged to DRAM for AllGather
                        tt = spool.tile([P, D], BF16, tag="tt")
                        nc.scalar.copy(out=tt[:], in_=psT[:])
                        nc.sync.dma_start(
                            out=cc_in[b * P : (b + 1) * P, :], in_=tt[:]
                        )

            phase_a(agat_s, va_s, KA_src, invc_s_t, wout_t, False)
            nc.gpsimd.collective_compute(
                "AllGather", mybir.AluOpType.bypass,
                replica_groups=[list(range(C))],
                ins=[cc_in.opt()], outs=[cc_out.opt()],
            )
            phase_a(agat_d, va_d, KA_dst, invc_d_t, win_t, True)

            # ---- phase C ----
            s_global = 0
            for h in range(2):
                tbl_half = cc_out[h * HALFT : (h + 1) * HALFT, :]
                for _ in range(NSUP_H[h]):
                    s = s_global
                    s_global += 1
                    xT_t = pool.tile([P, SUP], BF16, tag="xT")
                    nc.sync.dma_start(out=xT_t[:], in_=xT_d[s])
                    srcv_t = spool.tile([1, SUP], BF16, tag="srcv")
                    nc.sync.dma_start(out=srcv_t[:], in_=srcv_d[s])
                    gix = spool.tile([P, SIDX], I16, tag="gix")
                    nc.sync.dma_start(out=gix[:], in_=gidx_d[s])
                    go = pool.tile([P, SUP], BF16, tag="go")
                    nc.gpsimd.dma_gather(
                        out_ap=go[:].rearrange("p (a e) -> p a e", a=1),
                        in_ap=tbl_half,
                        idxs_ap=gix[:],
                        num_idxs=SUP, num_idxs_reg=SUP, elem_size=D,
                        transpose=True, single_packet=False,
                    )
                    yo = pool.tile([P, SUP], BF16, tag="yo")
                    W = 4 * D  # 512-wide groups
                    for g in range(KC // 4):
                        slg = slice(g * W, (g + 1) * W)
                        psB = psumc.tile([P, W], F32, tag="qB")
                        nc.tensor.matmul(
                            psB[:], ones_t[:], srcv_t[:, slg], start=True, stop=True
                        )
                        ohT = spool.tile([P, W], BF16, tag="ohT")
                        nc.vector.tensor_scalar(
                            ohT[:], psB[:], iota_c[:, 0:1], None,
                            mybir.AluOpType.is_equal,
                        )
                        psA = psumc.tile([P, W], F32, tag="qA", bufs=3)
                        nc.tensor.matmul(
                            psA[:], wselfT_t[:], xT_t[:, slg], start=True, stop=False
                        )
                        for k in range(4):
                            t = g * 4 + k
                            blk = BLK[s * KC + t]
                            sk = slice(k * D, (k + 1) * D)
                            nc.tensor.matmul(
                                psA[:, sk], tinb[blk][:],
                                ohT[:, sk], start=False, stop=True,
                                skip_group_check=True,
                            )
                        st = spool.tile([P, W], BF16, tag="st")
                        nc.vector.tensor_add(st[:], psA[:], go[:, slg])
                        nc.scalar.activation(
                            yo[:, slg], st[:], PRELU,
                            bias=bb_t[:, 0:1], scale=1.0, alpha=0.2,
                        )
                    nc.sync.dma_start(out=y[s * P : (s + 1) * P, :], in_=yo[:])

    nc.compile()
    return nc


def prepare_inputs(edge_attr, edge_index, W_self_w, W_self_b, W_in_w, W_out_w):
    edge_attr = np.ascontiguousarray(edge_attr, dtype=np.float32)
    src = np.asarray(edge_index[0], dtype=np.int64)
    dst = np.asarray(edge_index[1], dtype=np.int64)

    win = np.ascontiguousarray(np.asarray(W_in_w, np.float32).T).astype(BF)
    wout = np.ascontiguousarray(np.asarray(W_out_w, np.float32).T).astype(BF)
    wselfT = np.ascontiguousarray(np.asarray(W_self_w, np.float32).T).astype(BF)
    bbcol = np.asarray(W_self_b, np.float32).reshape(P, 1)
    iota = np.tile(np.arange(P, dtype=np.float32)[None, :], (P, 1))
    iotac = np.arange(P, dtype=np.float32).reshape(P, 1)

    # ---- phase A dense buckets (same scheme as v1) ----
    def build_a(node_of_edge):
        core = node_of_edge // NPC
        local = node_of_edge - core * NPC
        inblk = (local & 127).astype(np.float32)
        key = (core * NB + (local >> 7)).astype(np.int64)
        order = np.argsort(key, kind="stable")
        cnts = np.bincount(key, minlength=C * NB)
        KA = max(1, int(np.ceil(cnts.max() / P)))
        starts = np.zeros(C * NB, dtype=np.int64)
        np.cumsum(cnts[:-1], out=starts[1:])
        pos = np.arange(E, dtype=np.int64) - starts[key[order]]
        slot = key[order] * (P * KA) + pos
        agat = np.zeros((C * NB * P * KA, D), dtype=BF)
        agat[slot] = edge_attr[order].astype(BF)
        agat = agat.reshape(C, NB, P, KA * D)
        va = np.full((C * NB * P * KA), -1.0, dtype=np.float32)
        va[slot] = inblk[order]
        va = va.reshape(C, NB, P, KA)
        cnt_node = np.bincount(node_of_edge, minlength=N).astype(np.float32)
        inv = 1.0 / np.maximum(cnt_node, 1.0)
        inv_pad = np.zeros((C, NBP), dtype=np.float32)
        inv_pad[:, :NPC] = inv.reshape(C, NPC)
        invc = np.ascontiguousarray(inv_pad.reshape(C, NB, P).transpose(0, 2, 1))
        return KA, agat, va, invc

    KA_dst, agat_d, va_d, invc_d = build_a(dst)
    KA_src, agat_s, va_s, invc_s = build_a(src)

    # ---- phase C structure (uniform across cores) ----
    trow = lambda n: (n // NPC) * NBP + (n % NPC)
    rows_d = trow(dst)
    core_e = src // NPC
    src_loc = src - core_e * NPC
    blk_e = src_loc >> 7
    half_e = (rows_d >= HALFT).astype(np.int64)

    # per-core, per (half, block) counts -> uniform tile counts
    cnt = np.zeros((C, 2, NB), dtype=np.int64)
    for c in range(C):
        m = core_e == c
        np.add.at(cnt[c], (half_e[m], blk_e[m]), 1)
    maxcnt = cnt.max(axis=0)  # [2, NB]
    T_hb = np.maximum(1, np.ceil(maxcnt / P).astype(np.int64))  # tiles per (h,b)
    tiles_h = T_hb.sum(axis=1)
    NSUP_H = tuple(int(np.ceil(th / KC)) for th in tiles_h)
    NSUP = NSUP_H[0] + NSUP_H[1]
    NT = NSUP * KC
    # per-tile block ids (uniform): concat [b]*T_hb then pad to supertile mult
    BLK = []
    tile_base = np.zeros((2, NB), dtype=np.int64)  # first tile of (h,b)
    tb = 0
    for h in range(2):
        for b in range(NB):
            tile_base[h, b] = tb
            BLK.extend([b] * int(T_hb[h, b]))
            tb += int(T_hb[h, b])
        pad = NSUP_H[h] * KC - (tb - (0 if h == 0 else NSUP_H[0] * KC))
        BLK.extend([0] * pad)
        tb += pad
    BLK = tuple(BLK)
    assert len(BLK) == NT

    def wrap_idx(vals):
        S = len(vals) // 16
        t = np.zeros((16, S), dtype=np.int16)
        t[np.arange(len(vals)) % 16, np.arange(len(vals)) // 16] = vals.astype(
            np.int16
        )
        return np.tile(t, (8, 1))

    in_maps = []
    posts = []
    for c in range(C):
        m = core_e == c
        eids = np.nonzero(m)[0]
        # sort by (half, block, dst)
        okey = (half_e[eids] * NB + blk_e[eids]) * (2 * N) + dst[eids]
        order = np.argsort(okey, kind="stable")
        eids = eids[order]
        # slot assignment: per (h,b) segment starts at tile_base*P
        slot_edge = np.full(NT * P, -1, dtype=np.int64)
        pos = 0
        for h in range(2):
            for b in range(NB):
                k = int(cnt[c, h, b])
                base = int(tile_base[h, b]) * P
                slot_edge[base : base + k] = eids[pos : pos + k]
                pos += k
        assert pos == len(eids)
        valid = slot_edge >= 0
        ge = np.where(valid, slot_edge, 0)
        xs = np.where(valid[:, None], edge_attr[ge], 0).astype(BF)
        # [NT*P, D] -> [NSUP, P(d), SUP(cols)]
        xT = np.ascontiguousarray(
            xs.reshape(NSUP, SUP, D).transpose(0, 2, 1)
        )
        sv = np.where(valid, src_loc[ge] & 127, -1).astype(np.float32).astype(BF)
        sv = np.ascontiguousarray(sv.reshape(NSUP, 1, SUP))
        gi = np.where(valid, rows_d[ge] - half_e[ge] * HALFT, 0)
        gidx = np.stack(
            [wrap_idx(gi[s * SUP : (s + 1) * SUP]) for s in range(NSUP)]
        )
        in_maps.append(
            dict(
                agat_d=agat_d[c], va_d=va_d[c], agat_s=agat_s[c], va_s=va_s[c],
                invc_d=invc_d[c], invc_s=invc_s[c],
                xT=xT, srcv=sv, gidx=gidx,
                win=win, wout=wout, wselfT=wselfT, bbcol=bbcol,
                iota=iota, iotac=iotac,
            )
        )
        posts.append((slot_edge, valid))

    def postprocess(results):
        full = np.empty((E, D), dtype=np.float32)
        for c in range(C):
            slot_edge, valid = posts[c]
            yv = np.asarray(results[c]["y"]).astype(np.float32)
            yv = yv.reshape(NSUP, P, SUP).transpose(0, 2, 1).reshape(NT * P, D)
            full[slot_edge[valid]] = yv[valid]
        return full

    params = (KA_dst, KA_src, NSUP_H, BLK)
    return params, in_maps, postprocess


_NC_CACHE = {}


def run(inputs, trace=False, trace_kwargs=None):
    params, in_maps, post = prepare_inputs(
        inputs["edge_attr"],
        inputs["edge_index"],
        inputs["W_self_w"],
        inputs["W_self_b"],
        inputs["W_in_w"],
        inputs["W_out_w"],
    )
    key = params
    if key not in _NC_CACHE:
        _NC_CACHE[key] = build_kernel(*params)
    nc = _NC_CACHE[key]
    kw = {}
    if trace:
        kw["trace"] = True
        if trace_kwargs:
            kw.update(trace_kwargs)
    res = run_bass_kernel_spmd(nc, in_maps, core_ids=list(range(C)), **kw)
    return post(res.results), res


def kernel(**inputs) -> np.ndarray:
    out, _ = run(inputs)
    return out.astype(np.float32)


# revision 5
# speedup vs baseline: 1.2159x; 1.0323x over previous
"""DirectedEdgeConv Trainium2 kernel, 8-core SPMD — v2 (descriptor-free).

out[e] = prelu(x[e] @ Wself^T + b + T_in[src[e]] + T_out[dst[e]], 0.2)
  T_in  = scatter_mean(x, dst) @ Win^T    (bias folded into prelu's bias)
  T_out = scatter_mean(x, src) @ Wout^T

Sharding: edges assigned to core own(src). Per core:
  A1: stream dst-owner-grouped edges (dense KA buckets per 128-node
      block, bf16), one-hot PE scatter -> T_in[own nodes], finalized
      (x inv_cnt, @Win^T) and kept RESIDENT in SBUF (bf16, [n,d] blocks).
      No collective needed for T_in.
  A2: same for src-grouped edges -> T_out[own], finalized, transposed
      to [d,n] fp32 and written per-block to DRAM; one AllGather makes
      the full [d, 50176] table; relayout DMAs stage one half
      ([128, 25088] fp32, 98KB/partition) in SBUF at a time.
  C:  edges sorted by (dst_half, src_block, dst), padded per
      (half, block) to 128-multiples so the tile->block map is uniform
      across cores (SPMD).  Per 1536-edge supertile: one gpsimd
      ap_gather (SBUF->SBUF, 8 Q7 cores, no DMA descriptors) expands
      T_out^T columns; per 128-edge tile: ones-bcast matmul + is_equal
      builds the transposed one-hot, then 2 accumulated matmuls give
      psum[d,e] = Wself^T-mm(xT) + T_in-block-mm(ohT); DVE adds the
      gathered T_out^T; ACT applies Prelu(+bias).  y is written bf16,
      transposed [d, e]; the host de-permutes.
"""

import sys

sys.path.insert(0, "/opt/trn_rl_repo")

import numpy as np
import ml_dtypes

import concourse.bacc as bacc
import concourse.bass as bass
import concourse.mybir as mybir
import concourse.tile as tile
from concourse import library_config
from concourse.bass_utils import run_bass_kernel_spmd
from concourse.masks import make_identity

BF = ml_dtypes.bfloat16

P = 128
D = 128
C = 8
E = 600000
N = 50000
NPC = N // C            # 6250
NB = (NPC + P - 1) // P  # 49
NBP = NB * P            # 6272
TROWS = C * NBP         # 50176
HALFT = TROWS // 2      # 25088
KC = 24
SUP = P * KC            # 3072

F32 = mybir.dt.float32
BF16 = mybir.dt.bfloat16
I16 = mybir.dt.int16

PRELU = mybir.ActivationFunctionType.Prelu


def build_kernel(KA_dst, KA_src, NSUP_H, BLK):
    """NSUP_H: (nsup_half0, nsup_half1). BLK: tuple of per-tile block ids,
    len = 12*(nsup0+nsup1), uniform across cores."""
    NSUP = NSUP_H[0] + NSUP_H[1]
    SIDX = SUP // 16

    nc = bacc.Bacc(None, target_bir_lowering=False, debug=False)

    # ---- I/O ----
    agat_d = nc.dram_tensor("agat_d", [NB, P, KA_dst * D], BF16, kind="ExternalInput")
    va_d = nc.dram_tensor("va_d", [NB, P, KA_dst], F32, kind="ExternalInput")
    agat_s = nc.dram_tensor("agat_s", [NB, P, KA_src * D], BF16, kind="ExternalInput")
    va_s = nc.dram_tensor("va_s", [NB, P, KA_src], F32, kind="ExternalInput")
    invc_d = nc.dram_tensor("invc_d", [P, NB], F32, kind="ExternalInput")
    invc_s = nc.dram_tensor("invc_s", [P, NB], F32, kind="ExternalInput")
    xT_d = nc.dram_tensor("xT", [NSUP, P, SUP], BF16, kind="ExternalInput")
    srcv_d = nc.dram_tensor("srcv", [NSUP, 1, SUP], BF16, kind="ExternalInput")
    gidx_d = nc.dram_tensor("gidx", [NSUP, P, SIDX], I16, kind="ExternalInput")
    win = nc.dram_tensor("win", [D, D], BF16, kind="ExternalInput")      # W_in_w.T
    wout = nc.dram_tensor("wout", [D, D], BF16, kind="ExternalInput")    # W_out_w.T
    wselfT = nc.dram_tensor("wselfT", [D, D], BF16, kind="ExternalInput")  # W_self_w.T
    bbcol = nc.dram_tensor("bbcol", [P, 1], F32, kind="ExternalInput")   # W_self_b col
    iota_in = nc.dram_tensor("iota", [P, P], F32, kind="ExternalInput")
    iotac_in = nc.dram_tensor("iotac", [P, 1], F32, kind="ExternalInput")
    y = nc.dram_tensor("y", [NSUP * P, SUP], BF16, kind="ExternalOutput")

    with tile.TileContext(nc) as tc:
        with (
            tc.tile_pool(name="const", bufs=1) as cpool,
            tc.tile_pool(name="sbuf", bufs=4) as pool,
            tc.tile_pool(name="small", bufs=4) as spool,
            tc.tile_pool(name="psumA", bufs=1, space="PSUM") as psum,
            tc.tile_pool(name="psumC", bufs=2, space="PSUM") as psumc,
            tc.tile_pool(name="dram", bufs=1, space="DRAM") as dram,
        ):
            nc.gpsimd.load_library(library_config.mlp)
            # ---- constants ----
            ident = cpool.tile([P, P], BF16)
            make_identity(nc, ident[:])
            iota_t = cpool.tile([P, P], F32)
            nc.sync.dma_start(out=iota_t[:], in_=iota_in[:])
            iota_c = cpool.tile([P, 1], F32)
            nc.sync.dma_start(out=iota_c[:], in_=iotac_in[:])
            ones_t = cpool.tile([1, P], BF16)
            nc.vector.memset(ones_t[:], 1.0)
            win_t = cpool.tile([D, D], BF16)
            nc.sync.dma_start(out=win_t[:], in_=win[:])
            wout_t = cpool.tile([D, D], BF16)
            nc.sync.dma_start(out=wout_t[:], in_=wout[:])
            wselfT_t = cpool.tile([D, D], BF16)
            nc.sync.dma_start(out=wselfT_t[:], in_=wselfT[:])
            bb_t = cpool.tile([P, 1], F32)
            nc.sync.dma_start(out=bb_t[:], in_=bbcol[:])
            invc_d_t = cpool.tile([P, NB], F32)
            nc.sync.dma_start(out=invc_d_t[:], in_=invc_d[:])
            invc_s_t = cpool.tile([P, NB], F32)
            nc.sync.dma_start(out=invc_s_t[:], in_=invc_s[:])
            # resident T_in blocks [n_local, dout] bf16, one tile per block
            tinb = [cpool.tile([P, D], BF16, name=f"tinb{b}") for b in range(NB)]
            cc_in_a = dram.tile([25 * P, D], BF16)
            cc_in_b = dram.tile([24 * P, D], BF16)
            cc_out_a = dram.tile([C * 25 * P, D], BF16)
            cc_out_b = dram.tile([C * 24 * P, D], BF16)

            # ---- phase A ----
            def phase_a(agat, va, KA, invc_t, w_t, to_tin):
                for b in range(NB):
                    valt = spool.tile([P, KA], F32, tag="aval")
                    nc.sync.dma_start(out=valt[:], in_=va[b])
                    gat = pool.tile([P, KA * D], BF16, tag="agather")
                    nc.sync.dma_start(out=gat[:], in_=agat[b])
                    ps = psum.tile([P, D], F32, tag="pA")
                    for j in range(KA):
                        oh = spool.tile([P, P], BF16, tag="oh")
                        nc.vector.tensor_scalar(
                            oh[:], iota_t[:], valt[:, j : j + 1], None,
                            mybir.AluOpType.is_equal,
                        )
                        nc.tensor.matmul(
                            ps[:], oh[:], gat[:, j * D : (j + 1) * D],
                            start=(j == 0), stop=(j == KA - 1),
                        )
                    means = spool.tile([P, D], BF16, tag="means")
                    nc.vector.tensor_scalar(
                        means[:], ps[:], invc_t[:, b : b + 1], None,
                        mybir.AluOpType.mult,
                    )
                    pst = psum.tile([P, D], BF16, tag="pB")
                    nc.tensor.transpose(pst[:], means[:], ident[:])
                    meansT = spool.tile([P, D], BF16, tag="meansT")
                    nc.scalar.copy(out=meansT[:], in_=pst[:])
                    psT = psum.tile([P, D], F32, tag="pC")
                    nc.tensor.matmul(psT[:], meansT[:], w_t[:], start=True, stop=True)
                    if to_tin:
                        # [node, dout] -> resident bf16 block
                        nc.scalar.copy(out=tinb[b][:], in_=psT[:])
                    else:
                        # [node, dout] bf16 rows staged to DRAM for AllGather
                        tt = spool.tile([P, D], BF16, tag="tt")
                        nc.scalar.copy(out=tt[:], in_=psT[:])
                        if b < 25:
                            nc.sync.dma_start(
                                out=cc_in_a[b * P : (b + 1) * P, :], in_=tt[:]
                            )
                        else:
                            nc.sync.dma_start(
                                out=cc_in_b[(b - 25) * P : (b - 24) * P, :],
                                in_=tt[:],
                            )

            phase_a(agat_s, va_s, KA_src, invc_s_t, wout_t, False)
            nc.gpsimd.collective_compute(
                "AllGather", mybir.AluOpType.bypass,
                replica_groups=[list(range(C))],
                ins=[cc_in_a.opt()], outs=[cc_out_a.opt()],
            )
            nc.gpsimd.collective_compute(
                "AllGather", mybir.AluOpType.bypass,
                replica_groups=[list(range(C))],
                ins=[cc_in_b.opt()], outs=[cc_out_b.opt()],
            )
            phase_a(agat_d, va_d, KA_dst, invc_d_t, win_t, True)

            # ---- phase C ----
            s_global = 0
            for h in range(2):
                tbl_half = (cc_out_a if h == 0 else cc_out_b)[:, :]
                for _ in range(NSUP_H[h]):
                    s = s_global
                    s_global += 1
                    xT_t = pool.tile([P, SUP], BF16, tag="xT")
                    nc.sync.dma_start(out=xT_t[:], in_=xT_d[s])
                    srcv_t = spool.tile([1, SUP], BF16, tag="srcv")
                    nc.sync.dma_start(out=srcv_t[:], in_=srcv_d[s])
                    gix = spool.tile([P, SIDX], I16, tag="gix")
                    nc.sync.dma_start(out=gix[:], in_=gidx_d[s])
                    go = pool.tile([P, SUP], BF16, tag="go")
                    nc.gpsimd.dma_gather(
                        out_ap=go[:].rearrange("p (a e) -> p a e", a=1),
                        in_ap=tbl_half,
                        idxs_ap=gix[:],
                        num_idxs=SUP, num_idxs_reg=SUP, elem_size=D,
                        transpose=True, single_packet=False,
                    )
                    yo = pool.tile([P, SUP], BF16, tag="yo")
                    W = 4 * D  # 512-wide groups
                    for g in range(KC // 4):
                        slg = slice(g * W, (g + 1) * W)
                        psB = psumc.tile([P, W], F32, tag="qB")
                        nc.tensor.matmul(
                            psB[:], ones_t[:], srcv_t[:, slg], start=True, stop=True
                        )
                        ohT = spool.tile([P, W], BF16, tag="ohT")
                        nc.vector.tensor_scalar(
                            ohT[:], psB[:], iota_c[:, 0:1], None,
                            mybir.AluOpType.is_equal,
                        )
                        psA = psumc.tile([P, W], F32, tag="qA", bufs=3)
                        nc.tensor.matmul(
                            psA[:], wselfT_t[:], xT_t[:, slg], start=True, stop=False
                        )
                        for k in range(4):
                            t = g * 4 + k
                            blk = BLK[s * KC + t]
                            sk = slice(k * D, (k + 1) * D)
                            nc.tensor.matmul(
                                psA[:, sk], tinb[blk][:],
                                ohT[:, sk], start=False, stop=True,
                                skip_group_check=True,
                            )
                        st = spool.tile([P, W], BF16, tag="st")
                        nc.vector.tensor_add(st[:], psA[:], go[:, slg])
                        nc.scalar.activation(
                            yo[:, slg], st[:], PRELU,
                            bias=bb_t[:, 0:1], scale=1.0, alpha=0.2,
                        )
                    nc.sync.dma_start(out=y[s * P : (s + 1) * P, :], in_=yo[:])

    nc.compile()
    return nc


def prepare_inputs(edge_attr, edge_index, W_self_w, W_self_b, W_in_w, W_out_w):
    edge_attr = np.ascontiguousarray(edge_attr, dtype=np.float32)
    src = np.asarray(edge_index[0], dtype=np.int64)
    dst = np.asarray(edge_index[1], dtype=np.int64)

    win = np.ascontiguousarray(np.asarray(W_in_w, np.float32).T).astype(BF)
    wout = np.ascontiguousarray(np.asarray(W_out_w, np.float32).T).astype(BF)
    wselfT = np.ascontiguousarray(np.asarray(W_self_w, np.float32).T).astype(BF)
    bbcol = np.asarray(W_self_b, np.float32).reshape(P, 1)
    iota = np.tile(np.arange(P, dtype=np.float32)[None, :], (P, 1))
    iotac = np.arange(P, dtype=np.float32).reshape(P, 1)

    # ---- phase A dense buckets (same scheme as v1) ----
    def build_a(node_of_edge):
        core = node_of_edge // NPC
        local = node_of_edge - core * NPC
        inblk = (local & 127).astype(np.float32)
        key = (core * NB + (local >> 7)).astype(np.int64)
        order = np.argsort(key, kind="stable")
        cnts = np.bincount(key, minlength=C * NB)
        KA = max(1, int(np.ceil(cnts.max() / P)))
        starts = np.zeros(C * NB, dtype=np.int64)
        np.cumsum(cnts[:-1], out=starts[1:])
        pos = np.arange(E, dtype=np.int64) - starts[key[order]]
        slot = key[order] * (P * KA) + pos
        agat = np.zeros((C * NB * P * KA, D), dtype=BF)
        agat[slot] = edge_attr[order].astype(BF)
        agat = agat.reshape(C, NB, P, KA * D)
        va = np.full((C * NB * P * KA), -1.0, dtype=np.float32)
        va[slot] = inblk[order]
        va = va.reshape(C, NB, P, KA)
        cnt_node = np.bincount(node_of_edge, minlength=N).astype(np.float32)
        inv = 1.0 / np.maximum(cnt_node, 1.0)
        inv_pad = np.zeros((C, NBP), dtype=np.float32)
        inv_pad[:, :NPC] = inv.reshape(C, NPC)
        invc = np.ascontiguousarray(inv_pad.reshape(C, NB, P).transpose(0, 2, 1))
        return KA, agat, va, invc

    KA_dst, agat_d, va_d, invc_d = build_a(dst)
    KA_src, agat_s, va_s, invc_s = build_a(src)

    # ---- phase C structure (uniform across cores) ----
    HLOC = 25 * P  # 3200: halves split per-core locals at block 25
    core_d = dst // NPC
    loc_d = dst - core_d * NPC
    half_e = (loc_d >= HLOC).astype(np.int64)
    rows_half = np.where(
        half_e == 0, core_d * HLOC + loc_d,
        core_d * (NBP - HLOC) + (loc_d - HLOC),
    )
    core_e = src // NPC
    src_loc = src - core_e * NPC
    blk_e = src_loc >> 7

    # per-core, per (half, block) counts -> uniform tile counts
    cnt = np.zeros((C, 2, NB), dtype=np.int64)
    for c in range(C):
        m = core_e == c
        np.add.at(cnt[c], (half_e[m], blk_e[m]), 1)
    maxcnt = cnt.max(axis=0)  # [2, NB]
    T_hb = np.maximum(1, np.ceil(maxcnt / P).astype(np.int64))  # tiles per (h,b)
    tiles_h = T_hb.sum(axis=1)
    NSUP_H = tuple(int(np.ceil(th / KC)) for th in tiles_h)
    NSUP = NSUP_H[0] + NSUP_H[1]
    NT = NSUP * KC
    # per-tile block ids (uniform): concat [b]*T_hb then pad to supertile mult
    BLK = []
    tile_base = np.zeros((2, NB), dtype=np.int64)  # first tile of (h,b)
    tb = 0
    for h in range(2):
        for b in range(NB):
            tile_base[h, b] = tb
            BLK.extend([b] * int(T_hb[h, b]))
            tb += int(T_hb[h, b])
        pad = NSUP_H[h] * KC - (tb - (0 if h == 0 else NSUP_H[0] * KC))
        BLK.extend([0] * pad)
        tb += pad
    BLK = tuple(BLK)
    assert len(BLK) == NT

    def wrap_idx(vals):
        S = len(vals) // 16
        t = np.zeros((16, S), dtype=np.int16)
        t[np.arange(len(vals)) % 16, np.arange(len(vals)) // 16] = vals.astype(
            np.int16
        )
        return np.tile(t, (8, 1))

    in_maps = []
    posts = []
    for c in range(C):
        m = core_e == c
        eids = np.nonzero(m)[0]
        # sort by (half, block, dst)
        okey = (half_e[eids] * NB + blk_e[eids]) * (2 * N) + dst[eids]
        order = np.argsort(okey, kind="stable")
        eids = eids[order]
        # slot assignment: per (h,b) segment starts at tile_base*P
        slot_edge = np.full(NT * P, -1, dtype=np.int64)
        pos = 0
        for h in range(2):
            for b in range(NB):
                k = int(cnt[c, h, b])
                base = int(tile_base[h, b]) * P
                slot_edge[base : base + k] = eids[pos : pos + k]
                pos += k
        assert pos == len(eids)
        valid = slot_edge >= 0
        ge = np.where(valid, slot_edge, 0)
        xs = np.where(valid[:, None], edge_attr[ge], 0).astype(BF)
        # [NT*P, D] -> [NSUP, P(d), SUP(cols)]
        xT = np.ascontiguousarray(
            xs.reshape(NSUP, SUP, D).transpose(0, 2, 1)
        )
        sv = np.where(valid, src_loc[ge] & 127, -1).astype(np.float32).astype(BF)
        sv = np.ascontiguousarray(sv.reshape(NSUP, 1, SUP))
        gi = np.where(valid, rows_half[ge], 0)
        gidx = np.stack(
            [wrap_idx(gi[s * SUP : (s + 1) * SUP]) for s in range(NSUP)]
        )
        in_maps.append(
            dict(
                agat_d=agat_d[c], va_d=va_d[c], agat_s=agat_s[c], va_s=va_s[c],
                invc_d=invc_d[c], invc_s=invc_s[c],
                xT=xT, srcv=sv, gidx=gidx,
                win=win, wout=wout, wselfT=wselfT, bbcol=bbcol,
                iota=iota, iotac=iotac,
            )
        )
        posts.append((slot_edge, valid))

    def postprocess(results):
        full = np.empty((E, D), dtype=np.float32)
        for c in range(C):
            slot_edge, valid = posts[c]
            yv = np.asarray(results[c]["y"]).astype(np.float32)
            yv = yv.reshape(NSUP, P, SUP).transpose(0, 2, 1).reshape(NT * P, D)
            full[slot_edge[valid]] = yv[valid]
        return full

    params = (KA_dst, KA_src, NSUP_H, BLK)
    return params, in_maps, postprocess


_NC_CACHE = {}


def run(inputs, trace=False, trace_kwargs=None):
    params, in_maps, post = prepare_inputs(
        inputs["edge_attr"],
        inputs["edge_index"],
        inputs["W_self_w"],
        inputs["W_self_b"],
        inputs["W_in_w"],
        inputs["W_out_w"],
    )
    key = params
    if key not in _NC_CACHE:
        _NC_CACHE[key] = build_kernel(*params)
    nc = _NC_CACHE[key]
    kw = {}
    if trace:
        kw["trace"] = True
        if trace_kwargs:
            kw.update(trace_kwargs)
    res = run_bass_kernel_spmd(nc, in_maps, core_ids=list(range(C)), **kw)
    return post(res.results), res


def kernel(**inputs) -> np.ndarray:
    out, _ = run(inputs)
    return out.astype(np.float32)


# revision 6
# speedup vs baseline: 1.2442x; 1.0233x over previous
"""DirectedEdgeConv Trainium2 kernel, 8-core SPMD — v2 (descriptor-free).

out[e] = prelu(x[e] @ Wself^T + b + T_in[src[e]] + T_out[dst[e]], 0.2)
  T_in  = scatter_mean(x, dst) @ Win^T    (bias folded into prelu's bias)
  T_out = scatter_mean(x, src) @ Wout^T

Sharding: edges assigned to core own(src). Per core:
  A1: stream dst-owner-grouped edges (dense KA buckets per 128-node
      block, bf16), one-hot PE scatter -> T_in[own nodes], finalized
      (x inv_cnt, @Win^T) and kept RESIDENT in SBUF (bf16, [n,d] blocks).
      No collective needed for T_in.
  A2: same for src-grouped edges -> T_out[own], finalized, transposed
      to [d,n] fp32 and written per-block to DRAM; one AllGather makes
      the full [d, 50176] table; relayout DMAs stage one half
      ([128, 25088] fp32, 98KB/partition) in SBUF at a time.
  C:  edges sorted by (dst_half, src_block, dst), padded per
      (half, block) to 128-multiples so the tile->block map is uniform
      across cores (SPMD).  Per 1536-edge supertile: one gpsimd
      ap_gather (SBUF->SBUF, 8 Q7 cores, no DMA descriptors) expands
      T_out^T columns; per 128-edge tile: ones-bcast matmul + is_equal
      builds the transposed one-hot, then 2 accumulated matmuls give
      psum[d,e] = Wself^T-mm(xT) + T_in-block-mm(ohT); DVE adds the
      gathered T_out^T; ACT applies Prelu(+bias).  y is written bf16,
      transposed [d, e]; the host de-permutes.
"""

import sys

sys.path.insert(0, "/opt/trn_rl_repo")

import numpy as np
import ml_dtypes

import concourse.bacc as bacc
import concourse.bass as bass
import concourse.mybir as mybir
import concourse.tile as tile
from concourse import library_config
from concourse.bass_utils import run_bass_kernel_spmd
from concourse.masks import make_identity

BF = ml_dtypes.bfloat16

P = 128
D = 128
C = 8
E = 600000
N = 50000
NPC = N // C            # 6250
NB = (NPC + P - 1) // P  # 49
NBP = NB * P            # 6272
TROWS = C * NBP         # 50176
HALFT = TROWS // 2      # 25088
KC = 24
SUP = P * KC            # 3072

F32 = mybir.dt.float32
BF16 = mybir.dt.bfloat16
I16 = mybir.dt.int16

PRELU = mybir.ActivationFunctionType.Prelu


def build_kernel(KAB_d, OFF_d, KAB_s, OFF_s, NSUP_H, BLK):
    """NSUP_H: (nsup_half0, nsup_half1). BLK: tuple of per-tile block ids,
    len = 12*(nsup0+nsup1), uniform across cores."""
    NSUP = NSUP_H[0] + NSUP_H[1]
    SIDX = SUP // 16

    nc = bacc.Bacc(None, target_bir_lowering=False, debug=False)

    # ---- I/O ----
    SUMD, SUMS = int(OFF_d[-1]), int(OFF_s[-1])
    KAMAX = max(max(KAB_d), max(KAB_s))
    agat_d = nc.dram_tensor("agat_d", [P, SUMD * D], BF16, kind="ExternalInput")
    va_d = nc.dram_tensor("va_d", [P, SUMD], F32, kind="ExternalInput")
    agat_s = nc.dram_tensor("agat_s", [P, SUMS * D], BF16, kind="ExternalInput")
    va_s = nc.dram_tensor("va_s", [P, SUMS], F32, kind="ExternalInput")
    invc_d = nc.dram_tensor("invc_d", [P, NB], F32, kind="ExternalInput")
    invc_s = nc.dram_tensor("invc_s", [P, NB], F32, kind="ExternalInput")
    xT_d = nc.dram_tensor("xT", [NSUP, P, SUP], BF16, kind="ExternalInput")
    srcv_d = nc.dram_tensor("srcv", [NSUP, 1, SUP], BF16, kind="ExternalInput")
    gidx_d = nc.dram_tensor("gidx", [NSUP, P, SIDX], I16, kind="ExternalInput")
    win = nc.dram_tensor("win", [D, D], BF16, kind="ExternalInput")      # W_in_w.T
    wout = nc.dram_tensor("wout", [D, D], BF16, kind="ExternalInput")    # W_out_w.T
    wselfT = nc.dram_tensor("wselfT", [D, D], BF16, kind="ExternalInput")  # W_self_w.T
    bbcol = nc.dram_tensor("bbcol", [P, 1], F32, kind="ExternalInput")   # W_self_b col
    iota_in = nc.dram_tensor("iota", [P, P], BF16, kind="ExternalInput")
    iotac_in = nc.dram_tensor("iotac", [P, 1], F32, kind="ExternalInput")
    y = nc.dram_tensor("y", [NSUP * P, SUP], BF16, kind="ExternalOutput")

    with tile.TileContext(nc) as tc:
        with (
            tc.tile_pool(name="const", bufs=1) as cpool,
            tc.tile_pool(name="sbuf", bufs=4) as pool,
            tc.tile_pool(name="small", bufs=4) as spool,
            tc.tile_pool(name="psumA", bufs=1, space="PSUM") as psum,
            tc.tile_pool(name="psumC", bufs=2, space="PSUM") as psumc,
            tc.tile_pool(name="dram", bufs=1, space="DRAM") as dram,
        ):
            nc.gpsimd.load_library(library_config.mlp)
            # ---- constants ----
            ident = cpool.tile([P, P], BF16)
            make_identity(nc, ident[:])
            iota_t = cpool.tile([P, P], BF16)
            nc.sync.dma_start(out=iota_t[:], in_=iota_in[:])
            iota_c = cpool.tile([P, 1], F32)
            nc.sync.dma_start(out=iota_c[:], in_=iotac_in[:])
            ones_t = cpool.tile([1, P], BF16)
            nc.vector.memset(ones_t[:], 1.0)
            win_t = cpool.tile([D, D], BF16)
            nc.sync.dma_start(out=win_t[:], in_=win[:])
            wout_t = cpool.tile([D, D], BF16)
            nc.sync.dma_start(out=wout_t[:], in_=wout[:])
            wselfT_t = cpool.tile([D, D], BF16)
            nc.sync.dma_start(out=wselfT_t[:], in_=wselfT[:])
            bb_t = cpool.tile([P, 1], F32)
            nc.sync.dma_start(out=bb_t[:], in_=bbcol[:])
            invc_d_t = cpool.tile([P, NB], F32)
            nc.sync.dma_start(out=invc_d_t[:], in_=invc_d[:])
            invc_s_t = cpool.tile([P, NB], F32)
            nc.sync.dma_start(out=invc_s_t[:], in_=invc_s[:])
            # resident T_in blocks [n_local, dout] bf16, one tile per block
            tinb = [cpool.tile([P, D], BF16, name=f"tinb{b}") for b in range(NB)]
            cc_in_a = dram.tile([25 * P, D], BF16)
            cc_in_b = dram.tile([24 * P, D], BF16)
            cc_out_a = dram.tile([C * 25 * P, D], BF16)
            cc_out_b = dram.tile([C * 24 * P, D], BF16)

            # ---- phase A ----
            def phase_a(agat, va, KAB, OFF, invc_t, w_t, to_tin):
                for b in range(NB):
                    KA, off = int(KAB[b]), int(OFF[b])
                    valt = spool.tile([P, KAMAX], F32, tag="aval")
                    nc.sync.dma_start(out=valt[:, :KA], in_=va[:, off : off + KA])
                    gat = pool.tile([P, KAMAX * D], BF16, tag="agather")
                    nc.sync.dma_start(
                        out=gat[:, : KA * D],
                        in_=agat[:, off * D : (off + KA) * D],
                    )
                    ps = psum.tile([P, D], F32, tag="pA")
                    for j in range(KA):
                        oh = spool.tile([P, P], BF16, tag="oh")
                        nc.vector.tensor_scalar(
                            oh[:], iota_t[:], valt[:, j : j + 1], None,
                            mybir.AluOpType.is_equal,
                        )
                        nc.tensor.matmul(
                            ps[:], oh[:], gat[:, j * D : (j + 1) * D],
                            start=(j == 0), stop=(j == KA - 1),
                        )
                    means = spool.tile([P, D], BF16, tag="means")
                    nc.vector.tensor_scalar(
                        means[:], ps[:], invc_t[:, b : b + 1], None,
                        mybir.AluOpType.mult,
                    )
                    pst = psum.tile([P, D], BF16, tag="pB")
                    nc.tensor.transpose(pst[:], means[:], ident[:])
                    meansT = spool.tile([P, D], BF16, tag="meansT")
                    nc.scalar.copy(out=meansT[:], in_=pst[:])
                    psT = psum.tile([P, D], F32, tag="pC")
                    nc.tensor.matmul(psT[:], meansT[:], w_t[:], start=True, stop=True)
                    if to_tin:
                        # [node, dout] -> resident bf16 block
                        nc.scalar.copy(out=tinb[b][:], in_=psT[:])
                    else:
                        # [node, dout] bf16 rows staged to DRAM for AllGather
                        tt = spool.tile([P, D], BF16, tag="tt")
                        nc.scalar.copy(out=tt[:], in_=psT[:])
                        if b < 25:
                            nc.sync.dma_start(
                                out=cc_in_a[b * P : (b + 1) * P, :], in_=tt[:]
                            )
                        else:
                            nc.sync.dma_start(
                                out=cc_in_b[(b - 25) * P : (b - 24) * P, :],
                                in_=tt[:],
                            )

            phase_a(agat_s, va_s, KAB_s, OFF_s, invc_s_t, wout_t, False)
            nc.gpsimd.collective_compute(
                "AllGather", mybir.AluOpType.bypass,
                replica_groups=[list(range(C))],
                ins=[cc_in_a.opt()], outs=[cc_out_a.opt()],
            )
            nc.gpsimd.collective_compute(
                "AllGather", mybir.AluOpType.bypass,
                replica_groups=[list(range(C))],
                ins=[cc_in_b.opt()], outs=[cc_out_b.opt()],
            )
            phase_a(agat_d, va_d, KAB_d, OFF_d, invc_d_t, win_t, True)

            # ---- phase C ----
            s_global = 0
            for h in range(2):
                tbl_half = (cc_out_a if h == 0 else cc_out_b)[:, :]
                for _ in range(NSUP_H[h]):
                    s = s_global
                    s_global += 1
                    xT_t = pool.tile([P, SUP], BF16, tag="xT")
                    nc.sync.dma_start(out=xT_t[:], in_=xT_d[s])
                    srcv_t = spool.tile([1, SUP], BF16, tag="srcv")
                    nc.sync.dma_start(out=srcv_t[:], in_=srcv_d[s])
                    gix = spool.tile([P, SIDX], I16, tag="gix")
                    nc.sync.dma_start(out=gix[:], in_=gidx_d[s])
                    go = pool.tile([P, SUP], BF16, tag="go")
                    nc.gpsimd.dma_gather(
                        out_ap=go[:].rearrange("p (a e) -> p a e", a=1),
                        in_ap=tbl_half,
                        idxs_ap=gix[:],
                        num_idxs=SUP, num_idxs_reg=SUP, elem_size=D,
                        transpose=True, single_packet=False,
                    )
                    yo = pool.tile([P, SUP], BF16, tag="yo")
                    W = 4 * D  # 512-wide groups
                    for g in range(KC // 4):
                        slg = slice(g * W, (g + 1) * W)
                        psB = psumc.tile([P, W], F32, tag="qB")
                        nc.tensor.matmul(
                            psB[:], ones_t[:], srcv_t[:, slg], start=True, stop=True
                        )
                        ohT = spool.tile([P, W], BF16, tag="ohT")
                        nc.vector.tensor_scalar(
                            ohT[:], psB[:], iota_c[:, 0:1], None,
                            mybir.AluOpType.is_equal,
                        )
                        psA = psumc.tile([P, W], F32, tag="qA", bufs=3)
                        nc.tensor.matmul(
                            psA[:], wselfT_t[:], xT_t[:, slg], start=True, stop=False
                        )
                        for k in range(4):
                            t = g * 4 + k
                            blk = BLK[s * KC + t]
                            sk = slice(k * D, (k + 1) * D)
                            nc.tensor.matmul(
                                psA[:, sk], tinb[blk][:],
                                ohT[:, sk], start=False, stop=True,
                                skip_group_check=True,
                            )
                        st = spool.tile([P, W], BF16, tag="st")
                        nc.vector.tensor_add(st[:], psA[:], go[:, slg])
                        nc.scalar.activation(
                            yo[:, slg], st[:], PRELU,
                            bias=bb_t[:, 0:1], scale=1.0, alpha=0.2,
                        )
                    nc.sync.dma_start(out=y[s * P : (s + 1) * P, :], in_=yo[:])

    nc.compile()
    return nc


def prepare_inputs(edge_attr, edge_index, W_self_w, W_self_b, W_in_w, W_out_w):
    edge_attr = np.ascontiguousarray(edge_attr, dtype=np.float32)
    src = np.asarray(edge_index[0], dtype=np.int64)
    dst = np.asarray(edge_index[1], dtype=np.int64)

    win = np.ascontiguousarray(np.asarray(W_in_w, np.float32).T).astype(BF)
    wout = np.ascontiguousarray(np.asarray(W_out_w, np.float32).T).astype(BF)
    wselfT = np.ascontiguousarray(np.asarray(W_self_w, np.float32).T).astype(BF)
    bbcol = np.asarray(W_self_b, np.float32).reshape(P, 1)
    iota = np.tile(np.arange(P, dtype=np.float32)[None, :], (P, 1)).astype(BF)
    iotac = np.arange(P, dtype=np.float32).reshape(P, 1)

    # ---- phase A dense buckets (same scheme as v1) ----
    def build_a(node_of_edge):
        core = node_of_edge // NPC
        local = node_of_edge - core * NPC
        inblk = (local & 127).astype(np.float32)
        blk = (local >> 7).astype(np.int64)
        key = (core * NB + blk).astype(np.int64)
        order = np.argsort(key, kind="stable")
        cnts = np.bincount(key, minlength=C * NB)
        KAb = np.maximum(
            1, np.ceil(cnts.reshape(C, NB).max(axis=0) / P).astype(np.int64)
        )
        offs = np.zeros(NB + 1, dtype=np.int64)
        np.cumsum(KAb, out=offs[1:])
        SUM = int(offs[-1])
        starts = np.zeros(C * NB, dtype=np.int64)
        np.cumsum(cnts[:-1], out=starts[1:])
        pos = np.arange(E, dtype=np.int64) - starts[key[order]]
        b_o, c_o = blk[order], core[order]
        kae = KAb[b_o]
        p_o = pos // kae
        j_o = pos - p_o * kae
        flat = (c_o * P + p_o) * SUM + offs[b_o] + j_o
        agat = np.zeros((C * P * SUM, D), dtype=BF)
        agat[flat] = edge_attr[order].astype(BF)
        agat = agat.reshape(C, P, SUM * D)
        va = np.full((C * P * SUM), -1.0, dtype=np.float32)
        va[flat] = inblk[order]
        va = va.reshape(C, P, SUM)
        cnt_node = np.bincount(node_of_edge, minlength=N).astype(np.float32)
        inv = 1.0 / np.maximum(cnt_node, 1.0)
        inv_pad = np.zeros((C, NBP), dtype=np.float32)
        inv_pad[:, :NPC] = inv.reshape(C, NPC)
        invc = np.ascontiguousarray(inv_pad.reshape(C, NB, P).transpose(0, 2, 1))
        return tuple(KAb), tuple(offs), agat, va, invc

    KA_dst, OFF_dst, agat_d, va_d, invc_d = build_a(dst)
    KA_src, OFF_src, agat_s, va_s, invc_s = build_a(src)

    # ---- phase C structure (uniform across cores) ----
    HLOC = 25 * P  # 3200: halves split per-core locals at block 25
    core_d = dst // NPC
    loc_d = dst - core_d * NPC
    half_e = (loc_d >= HLOC).astype(np.int64)
    rows_half = np.where(
        half_e == 0, core_d * HLOC + loc_d,
        core_d * (NBP - HLOC) + (loc_d - HLOC),
    )
    core_e = src // NPC
    src_loc = src - core_e * NPC
    blk_e = src_loc >> 7

    # per-core, per (half, block) counts -> uniform tile counts
    cnt = np.zeros((C, 2, NB), dtype=np.int64)
    for c in range(C):
        m = core_e == c
        np.add.at(cnt[c], (half_e[m], blk_e[m]), 1)
    maxcnt = cnt.max(axis=0)  # [2, NB]
    T_hb = np.maximum(1, np.ceil(maxcnt / P).astype(np.int64))  # tiles per (h,b)
    tiles_h = T_hb.sum(axis=1)
    NSUP_H = tuple(int(np.ceil(th / KC)) for th in tiles_h)
    NSUP = NSUP_H[0] + NSUP_H[1]
    NT = NSUP * KC
    # per-tile block ids (uniform): concat [b]*T_hb then pad to supertile mult
    BLK = []
    tile_base = np.zeros((2, NB), dtype=np.int64)  # first tile of (h,b)
    tb = 0
    for h in range(2):
        for b in range(NB):
            tile_base[h, b] = tb
            BLK.extend([b] * int(T_hb[h, b]))
            tb += int(T_hb[h, b])
        pad = NSUP_H[h] * KC - (tb - (0 if h == 0 else NSUP_H[0] * KC))
        BLK.extend([0] * pad)
        tb += pad
    BLK = tuple(BLK)
    assert len(BLK) == NT

    def wrap_idx(vals):
        S = len(vals) // 16
        t = np.zeros((16, S), dtype=np.int16)
        t[np.arange(len(vals)) % 16, np.arange(len(vals)) // 16] = vals.astype(
            np.int16
        )
        return np.tile(t, (8, 1))

    in_maps = []
    posts = []
    for c in range(C):
        m = core_e == c
        eids = np.nonzero(m)[0]
        # sort by (half, block, dst)
        okey = (half_e[eids] * NB + blk_e[eids]) * (2 * N) + dst[eids]
        order = np.argsort(okey, kind="stable")
        eids = eids[order]
        # slot assignment: per (h,b) segment starts at tile_base*P
        slot_edge = np.full(NT * P, -1, dtype=np.int64)
        pos = 0
        for h in range(2):
            for b in range(NB):
                k = int(cnt[c, h, b])
                base = int(tile_base[h, b]) * P
                slot_edge[base : base + k] = eids[pos : pos + k]
                pos += k
        assert pos == len(eids)
        valid = slot_edge >= 0
        ge = np.where(valid, slot_edge, 0)
        xs = np.where(valid[:, None], edge_attr[ge], 0).astype(BF)
        # [NT*P, D] -> [NSUP, P(d), SUP(cols)]
        xT = np.ascontiguousarray(
            xs.reshape(NSUP, SUP, D).transpose(0, 2, 1)
        )
        sv = np.where(valid, src_loc[ge] & 127, -1).astype(np.float32).astype(BF)
        sv = np.ascontiguousarray(sv.reshape(NSUP, 1, SUP))
        gi = np.where(valid, rows_half[ge], 0)
        gidx = np.stack(
            [wrap_idx(gi[s * SUP : (s + 1) * SUP]) for s in range(NSUP)]
        )
        in_maps.append(
            dict(
                agat_d=agat_d[c], va_d=va_d[c], agat_s=agat_s[c], va_s=va_s[c],
                invc_d=invc_d[c], invc_s=invc_s[c],
                xT=xT, srcv=sv, gidx=gidx,
                win=win, wout=wout, wselfT=wselfT, bbcol=bbcol,
                iota=iota, iotac=iotac,
            )
        )
        posts.append((slot_edge, valid))

    def postprocess(results):
        full = np.empty((E, D), dtype=np.float32)
        for c in range(C):
            slot_edge, valid = posts[c]
            yv = np.asarray(results[c]["y"]).astype(np.float32)
            yv = yv.reshape(NSUP, P, SUP).transpose(0, 2, 1).reshape(NT * P, D)
            full[slot_edge[valid]] = yv[valid]
        return full

    params = (KA_dst, OFF_dst, KA_src, OFF_src, NSUP_H, BLK)
    return params, in_maps, postprocess


_NC_CACHE = {}


def run(inputs, trace=False, trace_kwargs=None):
    params, in_maps, post = prepare_inputs(
        inputs["edge_attr"],
        inputs["edge_index"],
        inputs["W_self_w"],
        inputs["W_self_b"],
        inputs["W_in_w"],
        inputs["W_out_w"],
    )
    key = params
    if key not in _NC_CACHE:
        _NC_CACHE[key] = build_kernel(*params)
    nc = _NC_CACHE[key]
    kw = {}
    if trace:
        kw["trace"] = True
        if trace_kwargs:
            kw.update(trace_kwargs)
    res = run_bass_kernel_spmd(nc, in_maps, core_ids=list(range(C)), **kw)
    return post(res.results), res


def kernel(**inputs) -> np.ndarray:
    out, _ = run(inputs)
    return out.astype(np.float32)
